# revision 1
# baseline (speedup 1.0000x reference)
"""Trainium2 Bass kernel for nn_DTFDynamicLayer (moe_routing dynamic-token
transformer layer), SPMD across 8 NeuronCores.

kernel(**inputs) takes FULL unsharded numpy inputs (keys as in setup_inputs)
and returns the FULL [B,T,D] output. Sharding happens inside:
  - router (scores/topk/positions): token-sharded + tiny AllGathers
  - packed sequence S=2048 split in 8 contiguous blocks of 256 (one per core)
  - K/V projections: tensor-parallel by heads (2 heads/core) + AllGather
  - Q projection, attention (all 16 heads), O-proj, residuals: local to the
    core's 256 packed positions
  - MLP: tensor-parallel over intermediate dim (704/core) over full S,
    partial sums combined with ReduceScatter back to own positions
All matmuls run as float32r (full PE rate).
"""
from contextlib import ExitStack

import numpy as np

import concourse.bass as bass
import concourse.mybir as mybir
import concourse.tile as tile
from concourse import bacc
from concourse.bass_utils import run_bass_kernel_spmd
from concourse.masks import make_identity

B, T, D = 2, 2048, 2048
H, HD = 16, 128
I = 5632
EPS = 1e-6
NC = 8
BT = B * T
TOKS = BT // NC          # 512 router tokens per core
K = T // 2               # 1024 selected per batch row
S = B * K                # 2048 packed tokens
SB = S // NC             # 256 packed slots per core
HPC = H // NC            # 2 heads per core
ICOL = I // NC           # 704
DC = D // 128            # 16
SCALE = 1.0 / float(np.sqrt(HD))
IC_CH = [128] * 5 + [ICOL - 5 * 128]   # I-col chunks per core: 5x128 + 64

F32 = mybir.dt.float32
F32R = mybir.dt.float32r
I32 = mybir.dt.int32
AF = mybir.ActivationFunctionType
OP = mybir.AluOpType
P = 128

_NC_CACHE = {}


def _rmsnorm_now(nc, pool, x, out, epst):
    """out = x * rsqrt(mean(x^2)+eps)  ([128, D] token-major, no weight)."""
    sq = pool.tile([P, D], F32, name="rn_sq")
    ssq = pool.tile([P, 1], F32, name="rn_ssq")
    nc.scalar.activation(sq[:], x[:], AF.Square, accum_out=ssq[:])
    rt = pool.tile([P, 1], F32, name="rn_rt")
    nc.scalar.activation(rt[:], ssq[:], AF.Sqrt, scale=1.0 / D,
                         bias=epst[:, :1])
    rec = pool.tile([P, 1], F32, name="rn_rec")
    nc.vector.reciprocal(rec[:], rt[:])
    nc.scalar.activation(out[:], x[:], AF.Copy, scale=rec[:, :1])


def _rope(nc, pool, q, cosT, sinm, out_ap, width):
    """q [128(hd), width] one head, feature-major. out = q*cos + rot(q)*sinm.
    rot(q)[0:64]=q[64:128], rot(q)[64:128]=q[0:64]; sinm rows 0:64 pre-negated.
    out_ap may be f32r (written via bitcast by caller passing f32 view)."""
    rot = pool.tile([P, width], F32, name="rp_rot", tag="rp_rot")
    nc.vector.tensor_copy(rot[0:64, :], q[64:P, :])
    nc.vector.tensor_copy(rot[64:P, :], q[0:64, :])
    t1 = pool.tile([P, width], F32, name="rp_t1", tag="rp_t1")
    nc.vector.tensor_mul(t1[:], q[:], cosT[:, :width])
    t2 = pool.tile([P, width], F32, name="rp_t2", tag="rp_t2")
    nc.vector.tensor_mul(t2[:], rot[:], sinm[:, :width])
    if out_ap.dtype == F32R:
        nc.vector.tensor_add(out_ap, t1[:].bitcast(F32R), t2[:].bitcast(F32R))
    else:
        nc.vector.tensor_add(out_ap, t1[:], t2[:])


def _gather_cossin_T(nc, pool, ppool, ident, cosf, sinf, rows_col, cosT, sinm,
                     col_off):
    """Gather cos/sin rows (128 of them, by rows_col int32 [128,1]) and write
    transposed into cosT/sinm at column offset col_off. sinm rows 0:64 negated.
    """
    for (src, dstT, negate) in ((cosf, cosT, False), (sinf, sinm, True)):
        g = pool.tile([P, HD], F32, name="cs_g", tag="cs_g")
        nc.gpsimd.indirect_dma_start(
            out=g[:], out_offset=None, in_=src[:],
            in_offset=bass.IndirectOffsetOnAxis(ap=rows_col, axis=0))
        pt = ppool.tile([P, P], F32, space="PSUM", name="cs_p", tag="cs_p")
        nc.tensor.transpose(pt[:], g[:], ident[:])
        sl = slice(col_off, col_off + P)
        if negate:
            nc.scalar.activation(dstT[0:64, sl], pt[0:64, :], AF.Copy,
                                 scale=-1.0)
            nc.scalar.activation(dstT[64:P, sl], pt[64:P, :], AF.Copy)
        else:
            nc.vector.tensor_copy(dstT[:, sl], pt[:])


def build(phases="full"):
    nc = bacc.Bacc(None, target_bir_lowering=False)
    _build(nc, phases)
    nc.finalize()
    return nc


def _build(nc, phases):
    dp = nc.declare_dram_parameter
    orig_s = dp("orig_s", [TOKS, D], F32, isOutput=False)
    post_s = dp("post_s", [TOKS, D], F32, isOutput=False)
    prior_s = dp("prior_s", [TOKS, D], F32, isOutput=False)
    hidden = dp("hidden", [BT, D], F32, isOutput=False)
    cosf = dp("cosf", [BT, HD], F32, isOutput=False)
    sinf = dp("sinf", [BT, HD], F32, isOutput=False)
    qw = dp("qw", [D, H * HD], F32, isOutput=False)
    kw_s = dp("kw_s", [D, HPC * HD], F32, isOutput=False)
    vw_s = dp("vw_s", [D, HPC * HD], F32, isOutput=False)
    qb = dp("qb", [H * HD, 1], F32, isOutput=False)
    kb_s = dp("kb_s", [HPC * HD, 1], F32, isOutput=False)
    vb_s = dp("vb_s", [HPC * HD, 1], F32, isOutput=False)
    ow = dp("ow", [H * HD, D], F32, isOutput=False)
    ln1w = dp("ln1w", [D, 1], F32, isOutput=False)
    ln2w = dp("ln2w", [D, 1], F32, isOutput=False)
    gatew_s = dp("gatew_s", [D, ICOL], F32, isOutput=False)
    upw_s = dp("upw_s", [D, ICOL], F32, isOutput=False)
    downw_s = dp("downw_s", [ICOL, D], F32, isOutput=False)
    # cconst: [beta_cu, beta_ce, beta_ce*ce_off, i0(=c*SB), unused,
    #          unused, i0row(=(c%4)*TOKS), b(=c//4)]
    cconst = dp("cconst", [1, 8], F32, isOutput=False)

    upd_out = dp("upd_out", [SB, D], F32, isOutput=True)
    selidx_out = dp("selidx_out", [SB, 1], I32, isOutput=True)
    dbg = dp("dbg", [P, 16], F32, isOutput=True)

    RG = [list(range(NC))]

    with tile.TileContext(nc) as tc, ExitStack() as es:
        # -------- DRAM internals (pool tiles => dep tracking) --------
        dr = es.enter_context(tc.tile_pool(name="dram", bufs=1, space="DRAM"))

        def dtile(name, shape, dtype=F32, shared=False):
            return dr.tile(shape, dtype, name=name,
                           addr_space="Shared" if shared else "Local")

        sc_in = dtile("sc_in", [TOKS, 1])
        sc_all = dtile("sc_all", [BT, 1], shared=True)
        mk_in = dtile("mk_in", [TOKS, 1])
        mk_all = dtile("mk_all", [BT, 1], shared=True)
        ps_in = dtile("ps_in", [TOKS, 1])
        ps_all = dtile("ps_all", [BT, 1], shared=True)
        selidx_d = dtile("selidx_d", [S + P, 1], I32)
        h1t_in = dtile("h1t_in", [D, SB])
        h1t_all = dtile("h1t_all", [NC * D, SB], shared=True)
        kf_in = dtile("kf_in", [HPC * HD, S])
        kf_all = dtile("kf_all", [H * HD, S], shared=True)
        vt_in = dtile("vt_in", [S, HPC * HD])
        vt_all = dtile("vt_all", [NC * S, HPC * HD], shared=True)
        h2t_in = dtile("h2t_in", [D, SB])
        h2t_all = dtile("h2t_all", [NC * D, SB], shared=True)
        mlp_in = dtile("mlp_in", [S, D])
        mlp_rs = dtile("mlp_rs", [SB, D])

        # -------- persistent SBUF --------
        pers = es.enter_context(tc.tile_pool(name="pers", bufs=1))
        ident = pers.tile([P, P], F32)
        make_identity(nc, ident[:])
        cc_sb = pers.tile([1, 8], F32)
        nc.sync.dma_start(out=cc_sb[:], in_=cconst[:])
        ccb = pers.tile([P, 8], F32)
        nc.gpsimd.partition_broadcast(ccb[:], cc_sb[:])
        col_bcu = ccb[:, 0:1]
        col_bce = ccb[:, 1:2]
        col_ceo = ccb[:, 2:3]
        col_i0 = ccb[:, 3:4]
        col_i0row = ccb[:, 6:7]
        col_b = ccb[:, 7:8]
        ones_r = pers.tile([P, 1], F32R)
        onef = pers.tile([P, 1], F32)
        nc.vector.memset(onef[:], 1.0)
        nc.vector.tensor_copy(ones_r[:], onef[:].bitcast(F32R))
        epst = pers.tile([P, 1], F32)
        nc.vector.memset(epst[:], EPS)
        iota_pf = pers.tile([P, SB], F32)      # value = p - f
        _it = pers.tile([P, SB], I32)
        nc.gpsimd.iota(_it[:], pattern=[[-1, SB]], base=0, channel_multiplier=1)
        nc.vector.tensor_copy(iota_pf[:], _it[:])
        iota_jmp = pers.tile([P, T], F32)      # value = j - p
        _it2 = pers.tile([P, T], I32)
        nc.gpsimd.iota(_it2[:], pattern=[[1, T]], base=0, channel_multiplier=-1)
        nc.vector.tensor_copy(iota_jmp[:], _it2[:])
        lnw_cols = pers.tile([P, 2 * DC], F32)  # [:, 0:16]=ln1, [:,16:32]=ln2
        nc.sync.dma_start(out=lnw_cols[:, 0:DC],
                          in_=ln1w.rearrange("(d p) one -> p d one", p=P))
        nc.sync.dma_start(out=lnw_cols[:, DC:2 * DC],
                          in_=ln2w.rearrange("(d p) one -> p d one", p=P))
        dbg_t = pers.tile([P, 16], F32)
        nc.vector.memset(dbg_t[:], 0.0)

        s_cols = [pers.tile([P, 1], F32, name=f"s_col{t}") for t in range(4)]
        m_cols = [pers.tile([P, 1], F32, name=f"m_col{t}") for t in range(4)]
        p_cols = [pers.tile([P, 1], F32, name=f"p_col{t}") for t in range(4)]

        # ============ Phase R1: scores for own 512 tokens ============
        with tc.tile_pool(name="router", bufs=2) as rp:
            for t in range(4):
                cu = rp.tile([P, 1], F32, name="cu")
                ce = rp.tile([P, 1], F32, name="ce")
                for (a_ap, b_ap, dst) in ((orig_s, post_s, cu),
                                          (post_s, prior_s, ce)):
                    at = rp.tile([P, D], F32, name="r_at")
                    bt = rp.tile([P, D], F32, name="r_bt")
                    nc.sync.dma_start(out=at[:], in_=a_ap[t * P:(t + 1) * P, :])
                    nc.sync.dma_start(out=bt[:], in_=b_ap[t * P:(t + 1) * P, :])
                    df = rp.tile([P, D], F32, name="r_df")
                    nc.vector.tensor_sub(df[:], at[:], bt[:])
                    sq = rp.tile([P, D], F32, name="r_sq")
                    ssq = rp.tile([P, 1], F32, name="r_ssq")
                    nc.scalar.activation(sq[:], df[:], AF.Square,
                                         accum_out=ssq[:])
                    nc.scalar.activation(dst[:], ssq[:], AF.Sqrt)
                t1 = rp.tile([P, 1], F32, name="r_t1")
                nc.vector.tensor_scalar(t1[:], cu[:], col_bcu, None,
                                        op0=OP.mult)
                nc.vector.scalar_tensor_tensor(
                    s_cols[t][:], in0=ce[:], scalar=col_bce, in1=t1[:],
                    op0=OP.mult, op1=OP.add)
                nc.vector.tensor_scalar(s_cols[t][:], s_cols[t][:], col_ceo,
                                        None, op0=OP.add)
            sc_flat = rp.tile([P, 4], F32, name="scflat")
            for t in range(4):
                nc.vector.tensor_copy(sc_flat[:, t:t + 1], s_cols[t][:])
            nc.sync.dma_start(
                out=sc_in.rearrange("(t p) one -> p t one", p=P),
                in_=sc_flat[:])
        nc.gpsimd.collective_compute("AllGather", OP.bypass, replica_groups=RG,
                                     ins=[sc_in[:]], outs=[sc_all[:]])

        if phases == "score":
            with tc.tile_pool(name="sfin", bufs=1) as fp:
                sall = fp.tile([P, BT // P], F32, name="sall")
                nc.sync.dma_start(
                    out=sall[:],
                    in_=sc_all.rearrange("(t p) one -> p t one", p=P))
                nc.vector.tensor_copy(dbg_t[:, 0:1], sall[:, 0:1])
                nc.vector.tensor_copy(dbg_t[:, 1:2], sall[:, 31:32])
                nc.vector.tensor_copy(dbg_t[:, 2:3], s_cols[0][:])
                nc.sync.dma_start(out=dbg[:], in_=dbg_t[:])
            return

        # ============ Phase R2: rank -> mask for own tokens ============
        # rank_i = #{j: s_j>s_i} + #{j<i: s_j==s_i} = (T - sum(le)) + sum(eq*jlt)
        # mask = rank <= K-1  <=>  acc = sum(le) - sum(eq*jlt) >= T-K+1
        with tc.tile_pool(name="rank", bufs=2) as rp:
            sbr = rp.tile([P, T], F32, name="sbr")
            _row_select_bcast(nc, rp, sc_all, col_b, sbr)
            for t in range(4):
                jlt = rp.tile([P, T], F32, name="k_jlt")
                rhs = rp.tile([P, 1], F32, name="k_rhs")
                nc.vector.tensor_scalar(rhs[:], col_i0row, float(t * P - 1),
                                        None, op0=OP.add)
                nc.vector.tensor_scalar(jlt[:], iota_jmp[:], rhs[:, :1], None,
                                        op0=OP.is_le)
                le = rp.tile([P, T], F32, name="k_le")
                nc.vector.tensor_scalar(le[:], sbr[:], s_cols[t][:, :1], None,
                                        op0=OP.is_le)
                eq = rp.tile([P, T], F32, name="k_eq")
                nc.vector.tensor_scalar(eq[:], sbr[:], s_cols[t][:, :1], None,
                                        op0=OP.is_equal)
                eqlt = rp.tile([P, T], F32, name="k_eqlt")
                nc.vector.tensor_mul(eqlt[:], eq[:], jlt[:])
                dif = rp.tile([P, T], F32, name="k_dif")
                nc.vector.tensor_sub(dif[:], le[:], eqlt[:])
                acc = rp.tile([P, 1], F32, name="k_acc")
                nc.vector.tensor_reduce(acc[:], dif[:],
                                        axis=mybir.AxisListType.X, op=OP.add)
                # mask = acc >= T-K+1  <=>  (-acc) <= -(T-K+1)
                nacc = rp.tile([P, 1], F32, name="k_nacc")
                nc.vector.tensor_scalar_mul(nacc[:], acc[:], -1.0)
                nc.vector.tensor_scalar(m_cols[t][:], nacc[:],
                                        float(-(T - K + 1)), None,
                                        op0=OP.is_le)
                if t == 0:
                    nc.vector.tensor_copy(dbg_t[:, 0:1], acc[:])
                    nc.vector.tensor_copy(dbg_t[:, 1:2], m_cols[t][:])
                    nc.vector.tensor_copy(dbg_t[:, 2:3], s_cols[t][:])
            mflat = rp.tile([P, 4], F32, name="mflat")
            for t in range(4):
                nc.vector.tensor_copy(mflat[:, t:t + 1], m_cols[t][:])
            nc.sync.dma_start(
                out=mk_in.rearrange("(t p) one -> p t one", p=P), in_=mflat[:])
        nc.gpsimd.collective_compute("AllGather", OP.bypass, replica_groups=RG,
                                     ins=[mk_in[:]], outs=[mk_all[:]])

        if phases == "rank":
            with tc.tile_pool(name="kfin", bufs=1) as fp:
                mall = fp.tile([P, BT // P], F32, name="mall")
                nc.sync.dma_start(
                    out=mall[:],
                    in_=mk_all.rearrange("(t p) one -> p t one", p=P))
                nc.vector.tensor_copy(dbg_t[:, 4:5], mall[:, 0:1])
                nc.vector.tensor_copy(dbg_t[:, 5:6], mall[:, 31:32])
                nc.sync.dma_start(out=dbg[:], in_=dbg_t[:])
            return

        # ============ Phase R3: positions ============
        with tc.tile_pool(name="pos", bufs=2) as rp:
            mbr = rp.tile([P, T], F32, name="mbr")
            _row_select_bcast(nc, rp, mk_all, col_b, mbr)
            for t in range(4):
                jlt = rp.tile([P, T], F32, name="p_jlt")
                rhs = rp.tile([P, 1], F32, name="p_rhs")
                nc.vector.tensor_scalar(rhs[:], col_i0row, float(t * P - 1),
                                        None, op0=OP.add)
                nc.vector.tensor_scalar(jlt[:], iota_jmp[:], rhs[:, :1], None,
                                        op0=OP.is_le)
                mj = rp.tile([P, T], F32, name="p_mj")
                nc.vector.tensor_mul(mj[:], mbr[:], jlt[:])
                nc.vector.tensor_reduce(p_cols[t][:], mj[:],
                                        axis=mybir.AxisListType.X, op=OP.add)
                if t == 0:
                    nc.vector.tensor_copy(dbg_t[:, 3:4], p_cols[t][:])
            pflat = rp.tile([P, 4], F32, name="pflat")
            for t in range(4):
                nc.vector.tensor_copy(pflat[:, t:t + 1], p_cols[t][:])
            nc.sync.dma_start(
                out=ps_in.rearrange("(t p) one -> p t one", p=P), in_=pflat[:])
        nc.gpsimd.collective_compute("AllGather", OP.bypass, replica_groups=RG,
                                     ins=[ps_in[:]], outs=[ps_all[:]])

        if phases == "pos":
            with tc.tile_pool(name="pfin", bufs=1) as fp:
                pall = fp.tile([P, BT // P], F32, name="pall")
                nc.sync.dma_start(
                    out=pall[:],
                    in_=ps_all.rearrange("(t p) one -> p t one", p=P))
                nc.vector.tensor_copy(dbg_t[:, 4:5], pall[:, 0:1])
                nc.vector.tensor_copy(dbg_t[:, 5:6], pall[:, 31:32])
                nc.sync.dma_start(out=dbg[:], in_=dbg_t[:])
            return

        # ============ Phase SCT: slot -> flat row map ============
        with tc.tile_pool(name="scat", bufs=4) as sp:
            zt = sp.tile([P, (S + P) // P], I32, name="sc_zero")
            nc.vector.memset(zt[:], 0)
            nc.sync.dma_start(
                out=selidx_d.rearrange("(t p) one -> p t one", p=P), in_=zt[:])
            mk_t = sp.tile([P, BT // P], F32, name="mk_t")
            ps_t = sp.tile([P, BT // P], F32, name="ps_t")
            nc.sync.dma_start(out=mk_t[:],
                              in_=mk_all.rearrange("(t p) one -> p t one", p=P))
            nc.sync.dma_start(out=ps_t[:],
                              in_=ps_all.rearrange("(t p) one -> p t one", p=P))
            dump_i = sp.tile([P, 1], I32, name="sc_dumpi")
            nc.gpsimd.iota(dump_i[:], pattern=[[0, 1]], base=S,
                           channel_multiplier=1)
            dump_f = sp.tile([P, 1], F32, name="sc_dumpf")
            nc.vector.tensor_copy(dump_f[:], dump_i[:])
            for t in range(BT // P):
                b = (t * P) // T
                # slot' = m*(pos + b*K - (S+p)) + (S+p)  (per-part dump row)
                t1 = sp.tile([P, 1], F32, name="sc_t1")
                nc.vector.tensor_scalar(t1[:], ps_t[:, t:t + 1],
                                        float(b * K), None, op0=OP.add)
                nc.vector.tensor_sub(t1[:], t1[:], dump_f[:])
                t2 = sp.tile([P, 1], F32, name="sc_t2")
                nc.vector.tensor_mul(t2[:], t1[:], mk_t[:, t:t + 1])
                nc.vector.tensor_add(t2[:], t2[:], dump_f[:])
                off_i = sp.tile([P, 1], I32, name="sc_off")
                nc.vector.tensor_copy(off_i[:], t2[:])
                val_i = sp.tile([P, 1], I32, name="sc_val")
                nc.gpsimd.iota(val_i[:], pattern=[[0, 1]], base=t * P,
                               channel_multiplier=1)
                nc.gpsimd.indirect_dma_start(
                    out=selidx_d[:],
                    out_offset=bass.IndirectOffsetOnAxis(ap=off_i[:, :1],
                                                         axis=0),
                    in_=val_i[:], in_offset=None)

        # ============ Phase G: gathers ============
        gpL = es.enter_context(tc.tile_pool(name="gpL", bufs=1))   # long-lived
        own_rows = []
        selh = []
        gate_g = []
        myslot = gpL.tile([P, 2], I32)
        _si = gpL.tile([P, 2], I32)
        _slotf = gpL.tile([P, 2], F32)
        for half in range(2):
            nc.gpsimd.iota(_si[:, half:half + 1], pattern=[[0, 1]],
                           base=half * P, channel_multiplier=1)
        nc.vector.tensor_copy(_slotf[:], _si[:])
        for half in range(2):
            nc.vector.tensor_scalar(_slotf[:, half:half + 1],
                                    _slotf[:, half:half + 1], col_i0, None,
                                    op0=OP.add)
        nc.vector.tensor_copy(myslot[:], _slotf[:])
        for half in range(2):
            orow = gpL.tile([P, 1], I32, name=f"orow{half}")
            nc.gpsimd.indirect_dma_start(
                out=orow[:], out_offset=None, in_=selidx_d[:],
                in_offset=bass.IndirectOffsetOnAxis(
                    ap=myslot[:, half:half + 1], axis=0))
            own_rows.append(orow)
            sh = gpL.tile([P, D], F32, name=f"selh{half}")
            nc.gpsimd.indirect_dma_start(
                out=sh[:], out_offset=None, in_=hidden[:],
                in_offset=bass.IndirectOffsetOnAxis(ap=orow[:, :1], axis=0),
                bounds_check=BT - 1, oob_is_err=False)
            selh.append(sh)
            ssc = gpL.tile([P, 1], F32, name=f"ssc{half}")
            nc.gpsimd.indirect_dma_start(
                out=ssc[:], out_offset=None, in_=sc_all[:],
                in_offset=bass.IndirectOffsetOnAxis(ap=orow[:, :1], axis=0))
            gg = gpL.tile([P, 1], F32, name=f"gate{half}")
            nc.scalar.activation(gg[:], ssc[:], AF.Sigmoid)
            gate_g.append(gg)
        x1 = [gpL.tile([P, D], F32, name=f"x1_{i}") for i in range(2)]

        if phases == "full":
            # attention-lived pool (opened before gpQ: LIFO close order)
            esA = ExitStack()
            gpA = esA.enter_context(tc.tile_pool(name="gpA", bufs=1))
            q_own = gpA.tile([P, H, SB], F32R)
            o_fm = gpA.tile([P, H, SB], F32R)

            # mid-lived pool: through QKV
            esQ = ExitStack()
            gpQ = esQ.enter_context(tc.tile_pool(name="gpQ", bufs=1))
            cosT_o = gpQ.tile([P, SB], F32)
            sinm_o = gpQ.tile([P, SB], F32)
            h1T_own = gpQ.tile([P, DC, SB], F32R)
            with tc.tile_pool(name="cso", bufs=3) as cp, \
                 tc.tile_pool(name="csop", bufs=4, space="PSUM") as cpp:
                for half in range(2):
                    _gather_cossin_T(nc, cp, cpp, ident, cosf, sinf,
                                     own_rows[half][:, :1], cosT_o, sinm_o,
                                     half * P)

            # ============ Phase N1: h1 = rmsnorm(selh); h1T own; AG ============
            with tc.tile_pool(name="n1", bufs=2) as np_, \
                 tc.tile_pool(name="n1p", bufs=4, space="PSUM") as npp:
                for half in range(2):
                    h1 = np_.tile([P, D], F32, name="h1")
                    _rmsnorm_now(nc, np_, selh[half], h1, epst)
                    for d in range(DC):
                        pt = npp.tile([P, P], F32, space="PSUM", name="n1_tp")
                        nc.tensor.transpose(pt[:], h1[:, d * P:(d + 1) * P],
                                            ident[:])
                        # fold ln1 weight (per-d-row) into the PSUM->SBUF copy
                        nc.vector.tensor_scalar(
                            h1T_own[:, d, half * P:(half + 1) * P],
                            pt[:].bitcast(F32R),
                            lnw_cols[:, d:d + 1], None, op0=OP.mult)
                for d in range(DC):
                    nc.sync.dma_start(out=h1t_in[d * P:(d + 1) * P, :],
                                      in_=h1T_own[:, d, :].bitcast(F32))
            nc.gpsimd.collective_compute("AllGather", OP.bypass, replica_groups=RG,
                                         ins=[h1t_in[:]], outs=[h1t_all[:]])

            # ============ Phase QKV ============
            with tc.tile_pool(name="qkv", bufs=1) as qp, \
                 tc.tile_pool(name="qkv2", bufs=2) as qp2, \
                 tc.tile_pool(name="qkvp", bufs=2, space="PSUM") as qpp:
                qb_sb = qp.tile([P, H], F32, name="qb_sb")
                nc.sync.dma_start(out=qb_sb[:],
                                  in_=qb.rearrange("(h p) one -> p h one", p=P))
                # ---- Q local: all heads, own 256 cols ----
                for grp in range(8):       # head groups of 2
                    qw_g = [qp.tile([P, 2 * P], F32R, name=f"qw_g{d}")
                            for d in range(DC)]
                    for d in range(DC):
                        nc.sync.dma_start(
                            out=qw_g[d][:],
                            in_=qw[d * P:(d + 1) * P,
                                   grp * 2 * P:(grp + 1) * 2 * P].bitcast(F32R))
                    for hh in range(2):
                        h = grp * 2 + hh
                        pt = qpp.tile([P, SB], F32, space="PSUM", name="q_ps")
                        for d in range(DC):
                            nc.tensor.matmul(pt[:], qw_g[d][:, hh * P:(hh + 1) * P],
                                             h1T_own[:, d, :], start=(d == 0),
                                             stop=(d == DC - 1))
                        qh = qp.tile([P, SB], F32, name="qh")
                        nc.scalar.activation(qh[:], pt[:], AF.Identity,
                                             bias=qb_sb[:, h:h + 1])
                        _rope(nc, qp, qh, cosT_o, sinm_o, q_own[:, h, :], SB)
                # ---- K,V TP by heads over full S (per 256-col block) ----
                kb_sb = qp.tile([P, HPC], F32, name="kb_sb")
                vb_sb = qp.tile([P, HPC], F32, name="vb_sb")
                nc.sync.dma_start(out=kb_sb[:],
                                  in_=kb_s.rearrange("(h p) one -> p h one", p=P))
                nc.sync.dma_start(out=vb_sb[:],
                                  in_=vb_s.rearrange("(h p) one -> p h one", p=P))
                kwt = [qp.tile([P, HPC * HD], F32R, name=f"kwt{d}")
                       for d in range(DC)]
                vwt = [qp.tile([P, HPC * HD], F32R, name=f"vwt{d}")
                       for d in range(DC)]
                for d in range(DC):
                    nc.sync.dma_start(out=kwt[d][:],
                                      in_=kw_s[d * P:(d + 1) * P, :].bitcast(F32R))
                    nc.sync.dma_start(out=vwt[d][:],
                                      in_=vw_s[d * P:(d + 1) * P, :].bitcast(F32R))
                allslot = qp.tile([P, S // P], I32, name="allslot")
                _af = qp.tile([P, S // P], I32, name="_af")
                for sc_ in range(S // P):
                    nc.gpsimd.iota(_af[:, sc_:sc_ + 1], pattern=[[0, 1]],
                                   base=sc_ * P, channel_multiplier=1)
                nc.vector.tensor_copy(allslot[:], _af[:])
                with tc.tile_pool(name="kvb", bufs=2) as kb_, \
                     tc.tile_pool(name="kvbp", bufs=2, space="PSUM") as kpp:
                    for sb_ in range(NC):
                        rhs = [qp.tile([P, SB], F32R, name=f"kv_rhs{d}")
                               for d in range(DC)]
                        for d in range(DC):
                            nc.sync.dma_start(
                                out=rhs[d][:],
                                in_=h1t_all[sb_ * D + d * P:sb_ * D + (d + 1) * P,
                                            :].bitcast(F32R))
                        # cos/sin for this block (2 x 128 slots)
                        csT = kb_.tile([P, SB], F32, name="csT")
                        snT = kb_.tile([P, SB], F32, name="snT")
                        rows_t = kb_.tile([P, 1], I32, name="rows_t")
                        for kk in range(2):
                            nc.gpsimd.indirect_dma_start(
                                out=rows_t[:], out_offset=None, in_=selidx_d[:],
                                in_offset=bass.IndirectOffsetOnAxis(
                                    ap=allslot[:, 2 * sb_ + kk:2 * sb_ + kk + 1],
                                    axis=0))
                            _gather_cossin_T(nc, kb_, kpp, ident, cosf, sinf,
                                             rows_t[:, :1], csT, snT, kk * P)
                        for hh in range(HPC):
                            # K
                            ptk = kpp.tile([P, SB], F32, space="PSUM", name="k_ps")
                            for d in range(DC):
                                nc.tensor.matmul(ptk[:],
                                                 kwt[d][:, hh * HD:(hh + 1) * HD],
                                                 rhs[d][:], start=(d == 0),
                                                 stop=(d == DC - 1))
                            kh = kb_.tile([P, SB], F32, name="kh")
                            nc.scalar.activation(kh[:], ptk[:], AF.Identity,
                                                 bias=kb_sb[:, hh:hh + 1])
                            kr = kb_.tile([P, SB], F32, name="kr")
                            _rope(nc, kb_, kh, csT, snT, kr[:], SB)
                            nc.sync.dma_start(
                                out=kf_in[hh * HD:(hh + 1) * HD,
                                          sb_ * SB:(sb_ + 1) * SB],
                                in_=kr[:])
                            # V
                            ptv = kpp.tile([P, SB], F32, space="PSUM", name="v_ps")
                            for d in range(DC):
                                nc.tensor.matmul(ptv[:],
                                                 vwt[d][:, hh * HD:(hh + 1) * HD],
                                                 rhs[d][:], start=(d == 0),
                                                 stop=(d == DC - 1))
                            vh = kb_.tile([P, SB], F32, name="vh")
                            nc.scalar.activation(vh[:], ptv[:], AF.Identity,
                                                 bias=vb_sb[:, hh:hh + 1])
                            for kk in range(2):
                                ptt = kpp.tile([P, P], F32, space="PSUM",
                                               name="vt_ps", tag="cs_p")
                                nc.tensor.transpose(
                                    ptt[:], vh[:, kk * P:(kk + 1) * P], ident[:])
                                vtt = kb_.tile([P, P], F32, name="vtt")
                                nc.vector.tensor_copy(vtt[:], ptt[:])
                                nc.sync.dma_start(
                                    out=vt_in[(2 * sb_ + kk) * P:
                                              (2 * sb_ + kk + 1) * P,
                                              hh * HD:(hh + 1) * HD],
                                    in_=vtt[:])
            esQ.close()
            nc.gpsimd.collective_compute("AllGather", OP.bypass, replica_groups=RG,
                                         ins=[kf_in[:]], outs=[kf_all[:]])
            nc.gpsimd.collective_compute("AllGather", OP.bypass, replica_groups=RG,
                                         ins=[vt_in[:]], outs=[vt_all[:]])

            # ============ Phase ATT ============
            with tc.tile_pool(name="att", bufs=2) as ap, \
                 tc.tile_pool(name="att1", bufs=1) as ap1, \
                 tc.tile_pool(name="attp", bufs=2, space="PSUM") as app:
                # causal masks per j-chunk: keep (p - f) <= i0 - jc*128
                masks = ap1.tile([P, S // P, SB], F32, name="masks")
                for jc in range(S // P):
                    rhsc = ap.tile([P, 1], F32, name="a_rhs")
                    nc.vector.tensor_scalar(rhsc[:], col_i0, float(-jc * P), None,
                                            op0=OP.add)
                    nc.vector.tensor_scalar(masks[:, jc, :], iota_pf[:],
                                            rhsc[:, :1], None, op0=OP.is_le)
                for h in range(H):
                    khead = ap.tile([P, S], F32R, name="khead")
                    nc.sync.dma_start(
                        out=khead[:],
                        in_=kf_all[h * P:(h + 1) * P, :].bitcast(F32R))
                    r, hh = h // HPC, h % HPC
                    vthead = ap.tile([P, S // P, P], F32R, name="vthead")
                    nc.sync.dma_start(
                        out=vthead[:],
                        in_=vt_all[r * S:(r + 1) * S,
                                   hh * HD:(hh + 1) * HD].rearrange(
                                       "(jc p) hd -> p jc hd", p=P).bitcast(F32R))
                    psum_o = app.tile([P, SB], F32, space="PSUM", name="a_po")
                    psum_s = app.tile([1, SB], F32, space="PSUM", name="a_ps")
                    for jc in range(S // P):
                        pa = app.tile([P, SB], F32, space="PSUM", name="a_pa")
                        nc.tensor.matmul(pa[:], khead[:, jc * P:(jc + 1) * P],
                                         q_own[:, h, :], start=True, stop=True)
                        et = ap.tile([P, SB], F32, name="a_et")
                        nc.scalar.activation(et[:], pa[:], AF.Exp, scale=SCALE)
                        ex = ap.tile([P, SB], F32R, name="a_ex")
                        nc.vector.tensor_mul(ex[:], et[:].bitcast(F32R),
                                             masks[:, jc, :].bitcast(F32R))
                        nc.tensor.matmul(psum_s[:], ones_r[:], ex[:],
                                         start=(jc == 0), stop=(jc == S // P - 1),
                                         skip_group_check=True)
                        nc.tensor.matmul(psum_o[:], vthead[:, jc, :], ex[:],
                                         start=(jc == 0), stop=(jc == S // P - 1),
                                         skip_group_check=True)
                    rec = ap.tile([1, SB], F32, name="a_rec")
                    nc.vector.reciprocal(rec[:], psum_s[:])
                    recb = ap.tile([P, SB], F32, name="a_recb")
                    nc.gpsimd.partition_broadcast(recb[:], rec[:])
                    nc.vector.tensor_mul(o_fm[:, h, :], psum_o[:].bitcast(F32R),
                                         recb[:].bitcast(F32R))

            # ============ Phase OPROJ: x1 = selh + ow.T @ o_fm ============
            with tc.tile_pool(name="opj", bufs=1) as op_, \
                 tc.tile_pool(name="opj2", bufs=2) as op2, \
                 tc.tile_pool(name="opjp", bufs=3, space="PSUM") as opp:
                for grp in range(4):      # D col groups of 512
                    ow_g = [op2.tile([P, 4 * P], F32R, name=f"ow_g{h}")
                            for h in range(H)]
                    for h in range(H):
                        nc.sync.dma_start(
                            out=ow_g[h][:],
                            in_=ow[h * P:(h + 1) * P,
                                   grp * 4 * P:(grp + 1) * 4 * P].bitcast(F32R))
                    for dd in range(4):
                        d = grp * 4 + dd
                        pt = opp.tile([P, SB], F32, space="PSUM", name="o_ps")
                        for h in range(H):
                            nc.tensor.matmul(pt[:], ow_g[h][:, dd * P:(dd + 1) * P],
                                             o_fm[:, h, :], start=(h == 0),
                                             stop=(h == H - 1))
                        for half in range(2):
                            ot = op_.tile([P, P], F32, name="o_sb")
                            nc.vector.tensor_copy(ot[:],
                                                  pt[:, half * P:(half + 1) * P])
                            pt2 = opp.tile([P, P], F32, space="PSUM", name="o_tp")
                            nc.tensor.transpose(pt2[:], ot[:], ident[:])
                            nc.vector.tensor_add(
                                x1[half][:, d * P:(d + 1) * P], pt2[:],
                                selh[half][:, d * P:(d + 1) * P])
            esA.close()

            # ============ Phase N2 + MLP ============
            with tc.tile_pool(name="mlp", bufs=1) as mp, \
                 tc.tile_pool(name="mlp2", bufs=2) as mp2, \
                 tc.tile_pool(name="mlpp", bufs=2, space="PSUM") as mpp:
                h2T_own = mp.tile([P, DC, SB], F32, name="h2T_own")
                for half in range(2):
                    h2 = mp.tile([P, D], F32, name="h2")
                    _rmsnorm_now(nc, mp, x1[half], h2, epst)
                    for d in range(DC):
                        pt = mpp.tile([P, P], F32, space="PSUM", name="m_tp")
                        nc.tensor.transpose(pt[:], h2[:, d * P:(d + 1) * P],
                                            ident[:])
                        nc.scalar.activation(
                            h2T_own[:, d, half * P:(half + 1) * P], pt[:],
                            AF.Copy, scale=lnw_cols[:, DC + d:DC + d + 1])
                for d in range(DC):
                    nc.sync.dma_start(out=h2t_in[d * P:(d + 1) * P, :],
                                      in_=h2T_own[:, d, :])
                nc.gpsimd.collective_compute(
                    "AllGather", OP.bypass, replica_groups=RG,
                    ins=[h2t_in[:]], outs=[h2t_all[:]])
                act_sb = [mp.tile([P, S], F32R, name=f"act{ic}")
                          for ic in range(len(IC_CH))]
                for sb_ in range(NC):
                    rhs = [mp.tile([P, SB], F32R, name=f"m_rhs{d}")
                           for d in range(DC)]
                    for d in range(DC):
                        nc.sync.dma_start(
                            out=rhs[d][:],
                            in_=h2t_all[sb_ * D + d * P:sb_ * D + (d + 1) * P,
                                        :].bitcast(F32R))
                    for ic, icw in enumerate(IC_CH):
                        gw = [mp.tile([P, P], F32R, name=f"m_gw{d}")
                              for d in range(DC)]
                        uw = [mp.tile([P, P], F32R, name=f"m_uw{d}")
                              for d in range(DC)]
                        for d in range(DC):
                            nc.sync.dma_start(
                                out=gw[d][:, :icw],
                                in_=gatew_s[d * P:(d + 1) * P,
                                            ic * P:ic * P + icw].bitcast(F32R))
                            nc.sync.dma_start(
                                out=uw[d][:, :icw],
                                in_=upw_s[d * P:(d + 1) * P,
                                          ic * P:ic * P + icw].bitcast(F32R))
                        ptg = mpp.tile([P, SB], F32, space="PSUM", name="m_ptg")
                        ptu = mpp.tile([P, SB], F32, space="PSUM", name="m_ptu")
                        for d in range(DC):
                            nc.tensor.matmul(ptg[:icw, :], gw[d][:, :icw],
                                             rhs[d][:], start=(d == 0),
                                             stop=(d == DC - 1))
                        for d in range(DC):
                            nc.tensor.matmul(ptu[:icw, :], uw[d][:, :icw],
                                             rhs[d][:], start=(d == 0),
                                             stop=(d == DC - 1))
                        sg = mp2.tile([P, SB], F32, name="m_sg")
                        nc.scalar.activation(sg[:icw, :], ptg[:icw, :], AF.Silu)
                        nc.vector.tensor_mul(
                            act_sb[ic][:icw, sb_ * SB:(sb_ + 1) * SB],
                            sg[:icw, :].bitcast(F32R),
                            ptu[:icw, :].bitcast(F32R))
                # down-proj partials -> mlp_in [S, D]
                for db in range(4):
                    dwt = [mp.tile([P, 4 * P], F32R, name=f"m_dw{ic}")
                           for ic in range(len(IC_CH))]
                    for ic, icw in enumerate(IC_CH):
                        nc.sync.dma_start(
                            out=dwt[ic][:icw, :],
                            in_=downw_s[ic * P:ic * P + icw,
                                        db * 4 * P:(db + 1) * 4 * P].bitcast(F32R))
                    for sc_ in range(S // P):
                        pt = mpp.tile([P, 4 * P], F32, space="PSUM", name="m_dps")
                        for ic, icw in enumerate(IC_CH):
                            nc.tensor.matmul(
                                pt[:], act_sb[ic][:icw, sc_ * P:(sc_ + 1) * P],
                                dwt[ic][:icw, :], start=(ic == 0),
                                stop=(ic == len(IC_CH) - 1))
                        mot = mp2.tile([P, 4 * P], F32, name="m_mot")
                        nc.vector.tensor_copy(mot[:], pt[:])
                        nc.sync.dma_start(
                            out=mlp_in[sc_ * P:(sc_ + 1) * P,
                                       db * 4 * P:(db + 1) * 4 * P],
                            in_=mot[:])
            nc.gpsimd.collective_compute("ReduceScatter", OP.add,
                                         replica_groups=RG, ins=[mlp_in[:]],
                                         outs=[mlp_rs[:]])

            # ============ Final ============
            with tc.tile_pool(name="fin", bufs=2) as fp:
                for half in range(2):
                    mt = fp.tile([P, D], F32, name="f_mt")
                    nc.sync.dma_start(out=mt[:],
                                      in_=mlp_rs[half * P:(half + 1) * P, :])
                    x2 = fp.tile([P, D], F32, name="f_x2")
                    nc.vector.tensor_add(x2[:], x1[half][:], mt[:])
                    dlt = fp.tile([P, D], F32, name="f_dlt")
                    nc.vector.tensor_sub(dlt[:], x2[:], selh[half][:])
                    upd = fp.tile([P, D], F32, name="f_upd")
                    nc.vector.scalar_tensor_tensor(
                        upd[:], in0=dlt[:], scalar=gate_g[half][:, :1],
                        in1=selh[half][:], op0=OP.mult, op1=OP.add)
                    nc.sync.dma_start(out=upd_out[half * P:(half + 1) * P, :],
                                      in_=upd[:])
                    nc.sync.dma_start(out=selidx_out[half * P:(half + 1) * P, :],
                                      in_=own_rows[half][:])
                nc.vector.tensor_copy(dbg_t[:, 8:9], gate_g[0][:])
                nc.sync.dma_start(out=dbg[:], in_=dbg_t[:])
        else:
            with tc.tile_pool(name="rfin", bufs=2) as fp:
                for half in range(2):
                    nc.sync.dma_start(
                        out=upd_out[half * P:(half + 1) * P, :],
                        in_=selh[half][:])
                    nc.sync.dma_start(
                        out=selidx_out[half * P:(half + 1) * P, :],
                        in_=own_rows[half][:])
                nc.vector.tensor_copy(dbg_t[:, 4:5], p_cols[0][:])
                nc.vector.tensor_copy(dbg_t[:, 5:6], gate_g[0][:])
                nc.sync.dma_start(out=dbg[:], in_=dbg_t[:])


def _row_select_bcast(nc, pool, src_all, col_b, out_bcast):
    """out = broadcast(src_all row-block b), b in {0,1} from col_b."""
    r0 = pool.tile([1, T], F32, name="rs_r0")
    r1 = pool.tile([1, T], F32, name="rs_r1")
    v = src_all.rearrange("(a t) one -> a (t one)", a=2)
    nc.sync.dma_start(out=r0[:], in_=v[0:1, :])
    nc.sync.dma_start(out=r1[:], in_=v[1:2, :])
    b0 = pool.tile([P, T], F32, name="rs_b0")
    b1 = pool.tile([P, T], F32, name="rs_b1")
    nc.gpsimd.partition_broadcast(b0[:], r0[:])
    nc.gpsimd.partition_broadcast(b1[:], r1[:])
    df = pool.tile([P, T], F32, name="rs_df")
    nc.vector.tensor_sub(df[:], b1[:], b0[:])
    nc.vector.scalar_tensor_tensor(out_bcast[:], in0=df[:], scalar=col_b,
                                   in1=b0[:], op0=OP.mult, op1=OP.add)


# =====================================================================
# Host side
# =====================================================================
def kernel(**inputs):
    hs = np.asarray(inputs["hidden_states"], np.float32)
    qw = np.asarray(inputs["q_w"], np.float32)
    kw = np.asarray(inputs["k_w"], np.float32)
    vw = np.asarray(inputs["v_w"], np.float32)
    bcu = float(np.asarray(inputs["beta_cu"]))
    bce = float(np.asarray(inputs["beta_ce"]))
    ceo = float(np.asarray(inputs["ce_off"]))

    hs_f = np.ascontiguousarray(hs.reshape(BT, D))
    orig_f = np.asarray(inputs["original"], np.float32).reshape(BT, D)
    post_f = np.asarray(inputs["posterior"], np.float32).reshape(BT, D)
    prior_f = np.asarray(inputs["prior"], np.float32).reshape(BT, D)
    cos_f = np.ascontiguousarray(
        np.asarray(inputs["cos"], np.float32).reshape(BT, HD))
    sin_f = np.ascontiguousarray(
        np.asarray(inputs["sin"], np.float32).reshape(BT, HD))

    gw = np.asarray(inputs["gate_w"], np.float32)
    uw = np.asarray(inputs["up_w"], np.float32)
    dw = np.asarray(inputs["down_w"], np.float32)

    in_maps = []
    for c in range(NC):
        sl = slice(c * TOKS, (c + 1) * TOKS)
        hd_sl = slice(c * HPC * HD, (c + 1) * HPC * HD)
        ic_sl = slice(c * ICOL, (c + 1) * ICOL)
        b = c // 4
        cconst = np.array([[bcu, bce, bce * ceo, c * SB, 0.0,
                            0.0, (c % 4) * TOKS, b]], np.float32)
        in_maps.append({
            "orig_s": np.ascontiguousarray(orig_f[sl]),
            "post_s": np.ascontiguousarray(post_f[sl]),
            "prior_s": np.ascontiguousarray(prior_f[sl]),
            "hidden": hs_f,
            "cosf": cos_f,
            "sinf": sin_f,
            "qw": qw,
            "kw_s": np.ascontiguousarray(kw[:, hd_sl]),
            "vw_s": np.ascontiguousarray(vw[:, hd_sl]),
            "qb": np.asarray(inputs["q_b"], np.float32).reshape(-1, 1),
            "kb_s": np.ascontiguousarray(
                np.asarray(inputs["k_b"], np.float32)[hd_sl]).reshape(-1, 1),
            "vb_s": np.ascontiguousarray(
                np.asarray(inputs["v_b"], np.float32)[hd_sl]).reshape(-1, 1),
            "ow": np.asarray(inputs["o_w"], np.float32),
            "ln1w": np.asarray(inputs["ln1_w"], np.float32).reshape(-1, 1),
            "ln2w": np.asarray(inputs["ln2_w"], np.float32).reshape(-1, 1),
            "gatew_s": np.ascontiguousarray(gw[:, ic_sl]),
            "upw_s": np.ascontiguousarray(uw[:, ic_sl]),
            "downw_s": np.ascontiguousarray(dw[ic_sl, :]),
            "cconst": cconst,
        })

    global _last_in_maps
    _last_in_maps = in_maps
    import os
    ph = os.environ.get("KPHASES", "full")
    if ph not in _NC_CACHE:
        _NC_CACHE[ph] = build(phases=ph)
    nc = _NC_CACHE[ph]
    res = run_bass_kernel_spmd(nc, in_maps, core_ids=list(range(NC)))

    global _last_results
    _last_results = [res.results[c] for c in range(NC)]
    out = hs_f.copy()
    for c in range(NC):
        idx = res.results[c]["selidx_out"][:, 0]
        out[idx] = res.results[c]["upd_out"]
    return out.reshape(B, T, D)


if __name__ == "__main__":
    import reference
    inp = {k: np.asarray(v) for k, v in reference.setup_inputs().items()}
    got = kernel(**inp)
    want = np.asarray(reference.reference(**reference.setup_inputs()))
    err = np.abs(got - want).max() / np.abs(want).max()
    print("rel err:", err)



# revision 10
# speedup vs baseline: 1.9697x; 1.9697x over previous
"""Trainium2 Bass kernel for nn_DTFDynamicLayer (dynamic-token transformer
layer), SPMD across 8 NeuronCores — optimized v2.

kernel(**inputs) takes FULL unsharded numpy inputs (keys as in setup_inputs)
and returns the FULL [B,T,D] output. Sharding strategy:
  - router (scores/rank): token-sharded (512 tokens/core) + 2 tiny AllGathers;
    slot positions computed locally via prefix-scan (no 3rd AllGather)
  - packed sequence S=2048; attention is HEAD-parallel: each core computes
    Q/K/V and full causal attention for its 2 heads over all S positions,
    then a partial O-projection combined with ReduceScatter
  - MLP tensor-parallel over intermediate dim (704/core), partials combined
    with ReduceScatter
  - all matmuls in bf16 (f32 PSUM accumulation); router stays f32
"""
from contextlib import ExitStack

import numpy as np
import ml_dtypes

import concourse.bass as bass
import concourse.mybir as mybir
import concourse.tile as tile
from concourse import bacc
from concourse.bass_utils import run_bass_kernel_spmd
from concourse.masks import make_identity

B, T, D = 2, 2048, 2048
H, HD = 16, 128
I = 5632
EPS = 1e-6
NC = 8
BT = B * T
TOKS = BT // NC          # 512 router tokens per core
K = T // 2               # 1024 selected per batch row
S = B * K                # 2048 packed tokens
SB = S // NC             # 256 packed slots per core
HPC = H // NC            # 2 heads per core
ICOL = I // NC           # 704
DC = D // 128            # 16
NIC = (ICOL + 127) // 128  # 6 intermediate chunks (5x128 + 64)
SCALE = 1.0 / float(np.sqrt(HD))

F32 = mybir.dt.float32
BF16 = mybir.dt.bfloat16
I32 = mybir.dt.int32
AF = mybir.ActivationFunctionType
OP = mybir.AluOpType
P = 128
NQ = 4                  # 512-wide column chunks of S
QW = S // NQ            # 512
BF16_NP = ml_dtypes.bfloat16

_NC_CACHE = {}


def _icw(ic):
    return min(P, ICOL - ic * P)


def build(phases="full"):
    nc = bacc.Bacc(None, target_bir_lowering=False)
    _build(nc, phases)
    nc.finalize()
    return nc


def _build(nc, phases):
    dp = nc.declare_dram_parameter
    orig_s = dp("orig_s", [TOKS, D], F32, isOutput=False)
    post_s = dp("post_s", [TOKS, D], F32, isOutput=False)
    prior_s = dp("prior_s", [TOKS, D], F32, isOutput=False)
    hidden = dp("hidden", [BT, D], F32, isOutput=False)
    cs_cat = dp("cs_cat", [BT, 2 * HD], BF16, isOutput=False)  # [cos | sinm]
    qw_s = dp("qw_s", [D, HPC * HD], BF16, isOutput=False)
    kw_s = dp("kw_s", [D, HPC * HD], BF16, isOutput=False)
    vw_s = dp("vw_s", [D, HPC * HD], BF16, isOutput=False)
    ow_s = dp("ow_s", [HPC * HD, D], BF16, isOutput=False)
    ln1w = dp("ln1w", [D, 1], F32, isOutput=False)
    ln2w = dp("ln2w", [D, 1], F32, isOutput=False)
    gatew_s = dp("gatew_s", [D, ICOL], BF16, isOutput=False)
    upw_s = dp("upw_s", [D, ICOL], BF16, isOutput=False)
    downw_s = dp("downw_s", [ICOL, D], BF16, isOutput=False)
    # cconst: [beta_cu, beta_ce, beta_ce*ce_off, i0(=c*SB), 0, 0,
    #          i0row(=(c%4)*TOKS), b(=c//4)]
    cconst = dp("cconst", [1, 8], F32, isOutput=False)

    upd_out = dp("upd_out", [SB, D], F32, isOutput=True)
    x2_out = dp("x2_out", [SB, D], F32, isOutput=True)
    selidx_out = dp("selidx_out", [SB, 1], I32, isOutput=True)
    dbg = dp("dbg", [P, 16], F32, isOutput=True)

    RG = [list(range(NC))]

    with tile.TileContext(nc) as tc, ExitStack() as es:
        # -------- DRAM internals --------
        dr = es.enter_context(tc.tile_pool(name="dram", bufs=1, space="DRAM"))

        def dtile(name, shape, dtype=F32, shared=False):
            return dr.tile(shape, dtype, name=name,
                           addr_space="Shared" if shared else "Local")

        warm_in = dtile("warm_in", [1, 8])
        warm_all = dtile("warm_all", [NC, 8], shared=True)
        sc_in = dtile("sc_in", [TOKS, 1])
        sc_all = dtile("sc_all", [BT, 1], shared=True)
        mk_in = dtile("mk_in", [TOKS, 1])
        mk_all = dtile("mk_all", [BT, 1], shared=True)
        ps_d = dtile("ps_d", [BT, 1])
        selidx_d = dtile("selidx_d", [S + P, 1], I32)
        h1t_inA = dtile("h1t_inA", [D // 2, SB], BF16)
        h1t_inB = dtile("h1t_inB", [D // 2, SB], BF16)
        h1t_allA = dtile("h1t_allA", [NC * D // 2, SB], BF16, shared=True)
        h1t_allB = dtile("h1t_allB", [NC * D // 2, SB], BF16, shared=True)
        opartA = dtile("opartA", [S, D // 2], BF16)
        opartB = dtile("opartB", [S, D // 2], BF16)
        o_rsA = dtile("o_rsA", [SB, D // 2], BF16)
        o_rsB = dtile("o_rsB", [SB, D // 2], BF16)
        h2t_inA = dtile("h2t_inA", [D // 2, SB], BF16)
        h2t_inB = dtile("h2t_inB", [D // 2, SB], BF16)
        h2t_allA = dtile("h2t_allA", [NC * D // 2, SB], BF16, shared=True)
        h2t_allB = dtile("h2t_allB", [NC * D // 2, SB], BF16, shared=True)
        mlpA = dtile("mlpA", [S, D // 2], BF16)
        mlpB = dtile("mlpB", [S, D // 2], BF16)
        mlp_rsA = dtile("mlp_rsA", [SB, D // 2], BF16)
        mlp_rsB = dtile("mlp_rsB", [SB, D // 2], BF16)

        # -------- persistent SBUF --------
        pers = es.enter_context(tc.tile_pool(name="pers", bufs=1))
        ident_bf = pers.tile([P, P], BF16)
        make_identity(nc, ident_bf[:])
        cc_sb = pers.tile([1, 8], F32)
        nc.sync.dma_start(out=cc_sb[:], in_=cconst[:])
        ccb = pers.tile([P, 8], F32)
        nc.gpsimd.partition_broadcast(ccb[:], cc_sb[:])
        col_bcu = ccb[:, 0:1]
        col_bce = ccb[:, 1:2]
        col_ceo = ccb[:, 2:3]
        col_i0 = ccb[:, 3:4]
        col_i0row = ccb[:, 6:7]
        col_b = ccb[:, 7:8]
        ones_bf = pers.tile([P, 1], BF16)
        nc.vector.memset(ones_bf[:], 1.0)
        epst = pers.tile([P, 1], F32)
        nc.vector.memset(epst[:], EPS)
        lnw_cols = pers.tile([P, 2 * DC], F32)  # [:, 0:16]=ln1, [:,16:32]=ln2
        nc.sync.dma_start(out=lnw_cols[:, 0:DC],
                          in_=ln1w.rearrange("(d p) one -> p d one", p=P))
        nc.sync.dma_start(out=lnw_cols[:, DC:2 * DC],
                          in_=ln2w.rearrange("(d p) one -> p d one", p=P))
        # causal masks for diagonal 128x512 chunks: keep when
        # (f - p - off) >= 0, off = (jc - 4*qb)*128
        att_mask = pers.tile([P, 4, QW], BF16)
        for r in range(4):
            nc.gpsimd.memset(att_mask[:, r, :], 1.0)
            nc.gpsimd.affine_select(
                out=att_mask[:, r, :], in_=att_mask[:, r, :],
                compare_op=OP.is_ge, fill=0.0, base=-r * P,
                pattern=[[1, QW]], channel_multiplier=-1)
        # strict-upper 32x32 (k<c) with cross-batch-row block zeroed
        tri32 = pers.tile([32, 32], F32)
        nc.gpsimd.memset(tri32[:], 1.0)
        nc.gpsimd.affine_select(out=tri32[:], in_=tri32[:],
                                compare_op=OP.is_gt, fill=0.0, base=0,
                                pattern=[[1, 32]], channel_multiplier=-1)
        nc.vector.memset(tri32[0:16, 16:32], 0.0)
        dbg_t = pers.tile([P, 16], F32)
        nc.vector.memset(dbg_t[:], 0.0)

        s_cols = [pers.tile([P, 1], F32, name=f"s_col{t}") for t in range(4)]
        m_cols = [pers.tile([P, 1], F32, name=f"m_col{t}") for t in range(4)]

        # warm up the collective rings with a tiny AllGather ASAP
        wt = pers.tile([1, 8], F32)
        nc.vector.memset(wt[:], 1.0)
        nc.sync.dma_start(out=warm_in[:], in_=wt[:])
        nc.gpsimd.collective_compute("AllGather", OP.bypass, replica_groups=RG,
                                     ins=[warm_in[:]], outs=[warm_all[:]])

        # ============ Phase R1: scores for own 512 tokens ============
        with tc.tile_pool(name="router", bufs=2) as rp:
            for t in range(4):
                cu = rp.tile([P, 1], F32, name="cu")
                ce = rp.tile([P, 1], F32, name="ce")
                for (a_ap, b_ap, dst) in ((orig_s, post_s, cu),
                                          (post_s, prior_s, ce)):
                    at = rp.tile([P, D], F32, name="r_at")
                    bt = rp.tile([P, D], F32, name="r_bt")
                    nc.sync.dma_start(out=at[:], in_=a_ap[t * P:(t + 1) * P, :])
                    nc.sync.dma_start(out=bt[:], in_=b_ap[t * P:(t + 1) * P, :])
                    df = rp.tile([P, D], F32, name="r_df")
                    nc.vector.tensor_sub(df[:], at[:], bt[:])
                    sq = rp.tile([P, D], F32, name="r_sq")
                    ssq = rp.tile([P, 1], F32, name="r_ssq")
                    nc.scalar.activation(sq[:], df[:], AF.Square,
                                         accum_out=ssq[:])
                    nc.scalar.activation(dst[:], ssq[:], AF.Sqrt)
                t1 = rp.tile([P, 1], F32, name="r_t1")
                nc.vector.tensor_scalar(t1[:], cu[:], col_bcu, None,
                                        op0=OP.mult)
                nc.vector.scalar_tensor_tensor(
                    s_cols[t][:], in0=ce[:], scalar=col_bce, in1=t1[:],
                    op0=OP.mult, op1=OP.add)
                nc.vector.tensor_scalar(s_cols[t][:], s_cols[t][:], col_ceo,
                                        None, op0=OP.add)
            sc_flat = rp.tile([P, 4], F32, name="scflat")
            for t in range(4):
                nc.vector.tensor_copy(sc_flat[:, t:t + 1], s_cols[t][:])
            nc.sync.dma_start(
                out=sc_in.rearrange("(t p) one -> p t one", p=P),
                in_=sc_flat[:])
        nc.gpsimd.collective_compute("AllGather", OP.bypass, replica_groups=RG,
                                     ins=[sc_in[:]], outs=[sc_all[:]])

        # ============ Phase R2: rank -> mask for own tokens ============
        # rank_i = #{j: s_j>s_i} + #{j<i: s_j==s_i}; mask = rank <= K-1
        # <=> acc = sum(le) - sum(eq*jlt) >= T-K+1
        with tc.tile_pool(name="rank1", bufs=1) as rp1, \
             tc.tile_pool(name="rank", bufs=2) as rp:
            iota_jmp = rp1.tile([P, T], F32, name="iota_jmp")  # value = j - p
            _it2 = rp1.tile([P, T], I32, name="iota_jmp_i")
            nc.gpsimd.iota(_it2[:], pattern=[[1, T]], base=0,
                           channel_multiplier=-1)
            nc.vector.tensor_copy(iota_jmp[:], _it2[:])
            sbr = rp1.tile([P, T], F32, name="sbr")
            _row_select_bcast(nc, rp1, sc_all, col_b, sbr)
            for t in range(4):
                jlt = rp.tile([P, T], F32, name="k_jlt")
                rhs = rp.tile([P, 1], F32, name="k_rhs")
                nc.vector.tensor_scalar(rhs[:], col_i0row, float(t * P - 1),
                                        None, op0=OP.add)
                nc.vector.tensor_scalar(jlt[:], iota_jmp[:], rhs[:, :1], None,
                                        op0=OP.is_le)
                le = rp.tile([P, T], F32, name="k_le")
                nc.vector.tensor_scalar(le[:], sbr[:], s_cols[t][:, :1], None,
                                        op0=OP.is_le)
                eq = rp.tile([P, T], F32, name="k_eq")
                nc.vector.tensor_scalar(eq[:], sbr[:], s_cols[t][:, :1], None,
                                        op0=OP.is_equal)
                eqlt = rp.tile([P, T], F32, name="k_eqlt")
                nc.vector.tensor_mul(eqlt[:], eq[:], jlt[:])
                dif = rp.tile([P, T], F32, name="k_dif")
                nc.vector.tensor_sub(dif[:], le[:], eqlt[:])
                acc = rp.tile([P, 1], F32, name="k_acc")
                nc.vector.tensor_reduce(acc[:], dif[:],
                                        axis=mybir.AxisListType.X, op=OP.add)
                nacc = rp.tile([P, 1], F32, name="k_nacc")
                nc.vector.tensor_scalar_mul(nacc[:], acc[:], -1.0)
                nc.vector.tensor_scalar(m_cols[t][:], nacc[:],
                                        float(-(T - K + 1)), None,
                                        op0=OP.is_le)
            mflat = rp.tile([P, 4], F32, name="mflat")
            for t in range(4):
                nc.vector.tensor_copy(mflat[:, t:t + 1], m_cols[t][:])
            nc.sync.dma_start(
                out=mk_in.rearrange("(t p) one -> p t one", p=P), in_=mflat[:])
        nc.gpsimd.collective_compute("AllGather", OP.bypass, replica_groups=RG,
                                     ins=[mk_in[:]], outs=[mk_all[:]])

        # ============ Phase R3: positions for ALL tokens (local) ============
        # layout [32 chunks (partition), 128 tokens (free)]; exclusive prefix
        # within chunk by shift+doubling; chunk offsets via tri32 matmul.
        with tc.tile_pool(name="pos", bufs=1) as pp, \
             tc.tile_pool(name="posp", bufs=1, space="PSUM") as ppp:
            mk_c = pp.tile([32, P], F32, name="mk_c")
            nc.sync.dma_start(out=mk_c[:],
                              in_=mk_all.rearrange("(c q) one -> c (q one)",
                                                   c=32))
            exA = pp.tile([32, P], F32, name="exA")
            exB = pp.tile([32, P], F32, name="exB")
            nc.vector.memset(exA[:, 0:1], 0.0)
            nc.vector.tensor_copy(exA[:, 1:P], mk_c[:, 0:P - 1])
            cur, nxt = exA, exB
            k = 1
            while k < P:
                nc.vector.tensor_copy(nxt[:, 0:k], cur[:, 0:k])
                nc.vector.tensor_add(nxt[:, k:P], cur[:, k:P], cur[:, 0:P - k])
                cur, nxt = nxt, cur
                k *= 2
            tot_col = pp.tile([32, 1], F32, name="tot_col")
            nc.vector.tensor_add(tot_col[:], cur[:, P - 1:P],
                                 mk_c[:, P - 1:P])
            ps_off = ppp.tile([32, 1], F32, space="PSUM", name="ps_off")
            nc.tensor.matmul(ps_off[:], tri32[:], tot_col[:], start=True,
                             stop=True)
            off_sb = pp.tile([32, 1], F32, name="off_sb")
            nc.vector.tensor_copy(off_sb[:], ps_off[:])
            pos_c = pp.tile([32, P], F32, name="pos_c")
            nc.vector.tensor_scalar(pos_c[:], cur[:], off_sb[:, :1], None,
                                    op0=OP.add)
            nc.sync.dma_start(
                out=ps_d.rearrange("(c q) one -> c (q one)", c=32),
                in_=pos_c[:])

        # ============ Phase SCT: slot -> flat row map ============
        with tc.tile_pool(name="scat", bufs=4) as sp:
            zt = sp.tile([P, (S + P) // P], I32, name="sc_zero")
            nc.vector.memset(zt[:], 0)
            nc.sync.dma_start(
                out=selidx_d.rearrange("(t p) one -> p t one", p=P), in_=zt[:])
            mk_t = sp.tile([P, BT // P], F32, name="mk_t")
            ps_t = sp.tile([P, BT // P], F32, name="ps_t")
            nc.sync.dma_start(out=mk_t[:],
                              in_=mk_all.rearrange("(t p) one -> p t one", p=P))
            nc.sync.dma_start(out=ps_t[:],
                              in_=ps_d.rearrange("(t p) one -> p t one", p=P))
            dump_i = sp.tile([P, 1], I32, name="sc_dumpi")
            nc.gpsimd.iota(dump_i[:], pattern=[[0, 1]], base=S,
                           channel_multiplier=1)
            dump_f = sp.tile([P, 1], F32, name="sc_dumpf")
            nc.vector.tensor_copy(dump_f[:], dump_i[:])
            for t in range(BT // P):
                b = (t * P) // T
                # slot' = m*(pos + b*K - (S+p)) + (S+p)
                t1 = sp.tile([P, 1], F32, name="sc_t1")
                nc.vector.tensor_scalar(t1[:], ps_t[:, t:t + 1],
                                        float(b * K), None, op0=OP.add)
                nc.vector.tensor_sub(t1[:], t1[:], dump_f[:])
                t2 = sp.tile([P, 1], F32, name="sc_t2")
                nc.vector.tensor_mul(t2[:], t1[:], mk_t[:, t:t + 1])
                nc.vector.tensor_add(t2[:], t2[:], dump_f[:])
                off_i = sp.tile([P, 1], I32, name="sc_off")
                nc.vector.tensor_copy(off_i[:], t2[:])
                val_i = sp.tile([P, 1], I32, name="sc_val")
                nc.gpsimd.iota(val_i[:], pattern=[[0, 1]], base=t * P,
                               channel_multiplier=1)
                nc.gpsimd.indirect_dma_start(
                    out=selidx_d[:],
                    out_offset=bass.IndirectOffsetOnAxis(ap=off_i[:, :1],
                                                         axis=0),
                    in_=val_i[:], in_offset=None)

        # ============ Phase G: gathers ============
        gpL = es.enter_context(tc.tile_pool(name="gpL", bufs=1))   # long-lived
        own_rows = []
        selh = []
        gate_g = []
        myslot = gpL.tile([P, 2], I32)
        _si = gpL.tile([P, 2], I32)
        _slotf = gpL.tile([P, 2], F32)
        for half in range(2):
            nc.gpsimd.iota(_si[:, half:half + 1], pattern=[[0, 1]],
                           base=half * P, channel_multiplier=1)
        nc.vector.tensor_copy(_slotf[:], _si[:])
        for half in range(2):
            nc.vector.tensor_scalar(_slotf[:, half:half + 1],
                                    _slotf[:, half:half + 1], col_i0, None,
                                    op0=OP.add)
        nc.vector.tensor_copy(myslot[:], _slotf[:])
        for half in range(2):
            orow = gpL.tile([P, 1], I32, name=f"orow{half}")
            nc.gpsimd.indirect_dma_start(
                out=orow[:], out_offset=None, in_=selidx_d[:],
                in_offset=bass.IndirectOffsetOnAxis(
                    ap=myslot[:, half:half + 1], axis=0))
            own_rows.append(orow)
            sh = gpL.tile([P, D], F32, name=f"selh{half}")
            nc.gpsimd.indirect_dma_start(
                out=sh[:], out_offset=None, in_=hidden[:],
                in_offset=bass.IndirectOffsetOnAxis(ap=orow[:, :1], axis=0),
                bounds_check=BT - 1, oob_is_err=False)
            selh.append(sh)
            ssc = gpL.tile([P, 1], F32, name=f"ssc{half}")
            nc.gpsimd.indirect_dma_start(
                out=ssc[:], out_offset=None, in_=sc_all[:],
                in_offset=bass.IndirectOffsetOnAxis(ap=orow[:, :1], axis=0))
            gg = gpL.tile([P, 1], F32, name=f"gate{half}")
            nc.scalar.activation(gg[:], ssc[:], AF.Sigmoid)
            gate_g.append(gg)
        x1 = [gpL.tile([P, D], F32, name=f"x1_{i}") for i in range(2)]
        sq_scr = gpL.tile([P, D], F32, name="sq_scr")

        if phases != "full":
            with tc.tile_pool(name="rfin", bufs=2) as fp:
                for half in range(2):
                    nc.sync.dma_start(
                        out=upd_out[half * P:(half + 1) * P, :],
                        in_=selh[half][:])
                    nc.sync.dma_start(
                        out=x2_out[half * P:(half + 1) * P, :],
                        in_=selh[half][:])
                    nc.sync.dma_start(out=selidx_out[half * P:(half + 1) * P, :],
                                      in_=own_rows[half][:])
                nc.vector.tensor_copy(dbg_t[:, 5:6], gate_g[0][:])
                nc.sync.dma_start(out=dbg[:], in_=dbg_t[:])
            return

        # ============ Phase N1: h1 = rmsnorm(selh)*ln1; AG feature-major ====
        with tc.tile_pool(name="n1", bufs=2) as np_, \
             tc.tile_pool(name="n1p", bufs=4, space="PSUM") as npp:
            h1T_own = np_.tile([P, DC, SB], BF16, name="h1T_own")
            for half in range(2):
                h1b = np_.tile([P, D], BF16, name="h1b")
                _rmsnorm_bf(nc, np_, selh[half], h1b, sq_scr, epst)
                for d in range(DC):
                    pt = npp.tile([P, P], BF16, space="PSUM", name="n1_tp")
                    nc.tensor.transpose(pt[:], h1b[:, d * P:(d + 1) * P],
                                        ident_bf[:])
                    nc.vector.tensor_scalar(
                        h1T_own[:, d, half * P:(half + 1) * P], pt[:],
                        lnw_cols[:, d:d + 1], None, op0=OP.mult)
            for d in range(DC):
                dst = h1t_inA if d < 8 else h1t_inB
                dd = d % 8
                nc.sync.dma_start(out=dst[dd * P:(dd + 1) * P, :],
                                  in_=h1T_own[:, d, :])
        nc.gpsimd.collective_compute("AllGather", OP.bypass, replica_groups=RG,
                                     ins=[h1t_inA[:]], outs=[h1t_allA[:]])
        nc.gpsimd.collective_compute("AllGather", OP.bypass, replica_groups=RG,
                                     ins=[h1t_inB[:]], outs=[h1t_allB[:]])

        # attention-lived pool (qh/kh/vtok/o_fm survive into OPROJ)
        esA = ExitStack()
        gpA = esA.enter_context(tc.tile_pool(name="gpA", bufs=1))
        qh = [gpA.tile([P, S], BF16, name=f"qh{h}") for h in range(HPC)]
        kh = [gpA.tile([P, S], BF16, name=f"kh{h}") for h in range(HPC)]
        vtok = [gpA.tile([P, S // P, HD], BF16, name=f"vtok{h}")
                for h in range(HPC)]
        o_fm = [gpA.tile([P, S], BF16, name=f"ofm{h}") for h in range(HPC)]
        ow_sb = gpA.tile([P, HPC, D], BF16, name="ow_sb")
        for h in range(HPC):
            nc.sync.dma_start(out=ow_sb[:, h, :],
                              in_=ow_s[h * P:(h + 1) * P, :])

        # ============ Phase QKV (own 2 heads, full S) ============
        esQ = ExitStack()
        gpQ = esQ.enter_context(tc.tile_pool(name="gpQ", bufs=1))
        h1T = gpQ.tile([P, DC, S], BF16, name="h1T")
        for d in range(DC):
            src = h1t_allA if d < 8 else h1t_allB
            dd = d % 8
            for j in range(NC):
                nc.sync.dma_start(
                    out=h1T[:, d, j * SB:(j + 1) * SB],
                    in_=src[j * (D // 2) + dd * P:j * (D // 2) + (dd + 1) * P,
                            :])
        qw_sb = gpQ.tile([P, DC, HPC * HD], BF16, name="qw_sb")
        kw_sb = gpQ.tile([P, DC, HPC * HD], BF16, name="kw_sb")
        vw_sb = gpQ.tile([P, DC, HPC * HD], BF16, name="vw_sb")
        for (wsb, wsrc) in ((qw_sb, qw_s), (kw_sb, kw_s), (vw_sb, vw_s)):
            for d in range(DC):
                nc.sync.dma_start(out=wsb[:, d, :],
                                  in_=wsrc[d * P:(d + 1) * P, :])
        # gather cos|sinm for all S slots, transposed to feature-major
        cosT = gpQ.tile([P, S], BF16, name="cosT")
        sinmT = gpQ.tile([P, S], BF16, name="sinmT")
        with tc.tile_pool(name="csg", bufs=3) as cp, \
             tc.tile_pool(name="csgp", bufs=4, space="PSUM") as cpp:
            allslot = cp.tile([P, S // P], I32, name="allslot")
            for sc_ in range(S // P):
                nc.gpsimd.iota(allslot[:, sc_:sc_ + 1], pattern=[[0, 1]],
                               base=sc_ * P, channel_multiplier=1)
            for sc_ in range(S // P):
                rows_t = cp.tile([P, 1], I32, name="rows_t")
                nc.gpsimd.indirect_dma_start(
                    out=rows_t[:], out_offset=None, in_=selidx_d[:],
                    in_offset=bass.IndirectOffsetOnAxis(
                        ap=allslot[:, sc_:sc_ + 1], axis=0))
                csg = cp.tile([P, 2 * HD], BF16, name="csg")
                nc.gpsimd.indirect_dma_start(
                    out=csg[:], out_offset=None, in_=cs_cat[:],
                    in_offset=bass.IndirectOffsetOnAxis(ap=rows_t[:, :1],
                                                        axis=0))
                for (lo, dstT) in ((0, cosT), (HD, sinmT)):
                    pt = cpp.tile([P, P], BF16, space="PSUM", name="cs_p")
                    nc.tensor.transpose(pt[:], csg[:, lo:lo + HD], ident_bf[:])
                    nc.vector.tensor_copy(dstT[:, sc_ * P:(sc_ + 1) * P],
                                          pt[:])

        with tc.tile_pool(name="qkv", bufs=3) as qp, \
             tc.tile_pool(name="qkvp", bufs=2, space="PSUM") as qpp:
            for h in range(HPC):
                for (wsb, dsth) in ((qw_sb, qh), (kw_sb, kh)):
                    for n in range(NQ):
                        pt = qpp.tile([P, QW], F32, space="PSUM", name="qk_ps")
                        for d in range(DC):
                            nc.tensor.matmul(
                                pt[:], wsb[:, d, h * HD:(h + 1) * HD],
                                h1T[:, d, n * QW:(n + 1) * QW],
                                start=(d == 0), stop=(d == DC - 1))
                        # rope: out = pt*cos + rot(pt)*sinm
                        rot = qp.tile([P, QW], F32, name="rp_rot")
                        nc.vector.tensor_copy(rot[0:64, :], pt[64:P, :])
                        nc.vector.tensor_copy(rot[64:P, :], pt[0:64, :])
                        t1 = qp.tile([P, QW], F32, name="rp_t1")
                        nc.vector.tensor_mul(
                            t1[:], pt[:], cosT[:, n * QW:(n + 1) * QW])
                        t2 = qp.tile([P, QW], F32, name="rp_t2")
                        nc.vector.tensor_mul(
                            t2[:], rot[:], sinmT[:, n * QW:(n + 1) * QW])
                        nc.vector.tensor_add(
                            dsth[h][:, n * QW:(n + 1) * QW], t1[:], t2[:])
                for n in range(NQ):
                    pt = qpp.tile([P, QW], F32, space="PSUM", name="v_ps")
                    for d in range(DC):
                        nc.tensor.matmul(
                            pt[:], vw_sb[:, d, h * HD:(h + 1) * HD],
                            h1T[:, d, n * QW:(n + 1) * QW],
                            start=(d == 0), stop=(d == DC - 1))
                    vsb = qp.tile([P, QW], BF16, name="v_sb")
                    nc.vector.tensor_copy(vsb[:], pt[:])
                    for kk in range(NQ):
                        ptt = qpp.tile([P, P], BF16, space="PSUM", name="vt_ps")
                        nc.tensor.transpose(ptt[:], vsb[:, kk * P:(kk + 1) * P],
                                            ident_bf[:])
                        nc.vector.tensor_copy(vtok[h][:, n * NQ + kk, :],
                                              ptt[:])
        esQ.close()

        # ============ Phase ATT (own heads, causal, full S queries) ========
        with tc.tile_pool(name="att", bufs=4) as ap, \
             tc.tile_pool(name="attpa", bufs=3, space="PSUM") as apa, \
             tc.tile_pool(name="attpo", bufs=2, space="PSUM") as apo:
            for h in range(HPC):
                for qb in range(NQ):
                    jmax = 4 * (qb + 1)
                    po = apo.tile([P, QW], F32, space="PSUM", name="a_po")
                    psum = apo.tile([1, QW], F32, space="PSUM", name="a_ps")
                    for jc in range(jmax):
                        pa = apa.tile([P, QW], F32, space="PSUM", name="a_pa")
                        nc.tensor.matmul(pa[:], kh[h][:, jc * P:(jc + 1) * P],
                                         qh[h][:, qb * QW:(qb + 1) * QW],
                                         start=True, stop=True)
                        et = ap.tile([P, QW], BF16, name="a_et")
                        nc.scalar.activation(et[:], pa[:], AF.Exp, scale=SCALE)
                        if jc >= 4 * qb:
                            nc.vector.tensor_mul(et[:], et[:],
                                                 att_mask[:, jc - 4 * qb, :])
                        nc.tensor.matmul(psum[:], ones_bf[:], et[:],
                                         start=(jc == 0), stop=(jc == jmax - 1),
                                         skip_group_check=True)
                        nc.tensor.matmul(po[:], vtok[h][:, jc, :], et[:],
                                         start=(jc == 0), stop=(jc == jmax - 1),
                                         skip_group_check=True)
                    rec = ap.tile([1, QW], F32, name="a_rec")
                    nc.vector.reciprocal(rec[:], psum[:])
                    recb = ap.tile([P, QW], F32, name="a_recb")
                    nc.gpsimd.partition_broadcast(recb[:], rec[:])
                    nc.vector.tensor_mul(o_fm[h][:, qb * QW:(qb + 1) * QW],
                                         po[:], recb[:])

        # ============ Phase OPROJ: opart[s,d] = sum_h o_fm_h.T @ ow_h ======
        with tc.tile_pool(name="opj", bufs=3) as op_, \
             tc.tile_pool(name="opjp", bufs=3, space="PSUM") as opp:
            for nd in range(NQ):
                dstpart = opartA if nd < 2 else opartB
                dlo = (nd % 2) * QW
                for qc in range(S // P):
                    pt = opp.tile([P, QW], F32, space="PSUM", name="o_ps")
                    for h in range(HPC):
                        nc.tensor.matmul(
                            pt[:], o_fm[h][:, qc * P:(qc + 1) * P],
                            ow_sb[:, h, nd * QW:(nd + 1) * QW],
                            start=(h == 0), stop=(h == HPC - 1))
                    osb = op_.tile([P, QW], BF16, name="o_sb")
                    nc.vector.tensor_copy(osb[:], pt[:])
                    nc.sync.dma_start(
                        out=dstpart[qc * P:(qc + 1) * P, dlo:dlo + QW],
                        in_=osb[:])
                if nd == 1:
                    nc.gpsimd.collective_compute(
                        "ReduceScatter", OP.add, replica_groups=RG,
                        ins=[opartA[:]], outs=[o_rsA[:]])
            nc.gpsimd.collective_compute(
                "ReduceScatter", OP.add, replica_groups=RG,
                ins=[opartB[:]], outs=[o_rsB[:]])
        esA.close()

        # MLP weights (loads overlap RS_o / N2 / AG_h2)
        esM = ExitStack()
        gpMw = esM.enter_context(tc.tile_pool(name="gpMw", bufs=1))
        gw_sb = gpMw.tile([P, DC, ICOL], BF16, name="gw_sb")
        uw_sb = gpMw.tile([P, DC, ICOL], BF16, name="uw_sb")
        dw_sb = gpMw.tile([P, NIC, D], BF16, name="dw_sb")
        for (wsb, wsrc) in ((gw_sb, gatew_s), (uw_sb, upw_s)):
            for d in range(DC):
                nc.scalar.dma_start(out=wsb[:, d, :],
                                    in_=wsrc[d * P:(d + 1) * P, :])
        for ic in range(NIC):
            icw = _icw(ic)
            nc.scalar.dma_start(out=dw_sb[0:icw, ic, :],
                                in_=downw_s[ic * P:ic * P + icw, :])

        # ============ Phase N2 + AG ============
        with tc.tile_pool(name="n2", bufs=2) as np2, \
             tc.tile_pool(name="n2p", bufs=4, space="PSUM") as npp2:
            h2T_own = np2.tile([P, DC, SB], BF16, name="h2T_own")
            for half in range(2):
                orsa = np2.tile([P, D // 2], BF16, name="orsa")
                orsb = np2.tile([P, D // 2], BF16, name="orsb")
                nc.sync.dma_start(out=orsa[:],
                                  in_=o_rsA[half * P:(half + 1) * P, :])
                nc.sync.dma_start(out=orsb[:],
                                  in_=o_rsB[half * P:(half + 1) * P, :])
                nc.vector.tensor_add(x1[half][:, 0:D // 2], selh[half][:, 0:D // 2],
                                     orsa[:])
                nc.vector.tensor_add(x1[half][:, D // 2:D],
                                     selh[half][:, D // 2:D], orsb[:])
                h2b = np2.tile([P, D], BF16, name="h2b")
                _rmsnorm_bf(nc, np2, x1[half], h2b, sq_scr, epst)
                for d in range(DC):
                    pt = npp2.tile([P, P], BF16, space="PSUM", name="n2_tp")
                    nc.tensor.transpose(pt[:], h2b[:, d * P:(d + 1) * P],
                                        ident_bf[:])
                    nc.vector.tensor_scalar(
                        h2T_own[:, d, half * P:(half + 1) * P], pt[:],
                        lnw_cols[:, DC + d:DC + d + 1], None, op0=OP.mult)
            for d in range(DC):
                dst = h2t_inA if d < 8 else h2t_inB
                dd = d % 8
                nc.sync.dma_start(out=dst[dd * P:(dd + 1) * P, :],
                                  in_=h2T_own[:, d, :])
        nc.gpsimd.collective_compute("AllGather", OP.bypass, replica_groups=RG,
                                     ins=[h2t_inA[:]], outs=[h2t_allA[:]])
        nc.gpsimd.collective_compute("AllGather", OP.bypass, replica_groups=RG,
                                     ins=[h2t_inB[:]], outs=[h2t_allB[:]])

        # ============ Phase MLP (TP over I) ============
        gpMa = esM.enter_context(tc.tile_pool(name="gpMa", bufs=1))
        h2T = gpMa.tile([P, DC, S], BF16, name="h2T")
        for d in range(DC):
            src = h2t_allA if d < 8 else h2t_allB
            dd = d % 8
            for j in range(NC):
                nc.sync.dma_start(
                    out=h2T[:, d, j * SB:(j + 1) * SB],
                    in_=src[j * (D // 2) + dd * P:j * (D // 2) + (dd + 1) * P,
                            :])
        act_sb = gpMa.tile([P, NIC, S], BF16, name="act_sb")
        with tc.tile_pool(name="mlp", bufs=3) as mp, \
             tc.tile_pool(name="mlpp", bufs=2, space="PSUM") as mpp:
            for n in range(NQ):
                for ic in range(NIC):
                    icw = _icw(ic)
                    pg = mpp.tile([P, QW], F32, space="PSUM", name="m_pg")
                    pu = mpp.tile([P, QW], F32, space="PSUM", name="m_pu")
                    for d in range(DC):
                        nc.tensor.matmul(pg[0:icw, :],
                                         gw_sb[:, d, ic * P:ic * P + icw],
                                         h2T[:, d, n * QW:(n + 1) * QW],
                                         start=(d == 0), stop=(d == DC - 1))
                    for d in range(DC):
                        nc.tensor.matmul(pu[0:icw, :],
                                         uw_sb[:, d, ic * P:ic * P + icw],
                                         h2T[:, d, n * QW:(n + 1) * QW],
                                         start=(d == 0), stop=(d == DC - 1))
                    sg = mp.tile([P, QW], BF16, name="m_sg")
                    nc.scalar.activation(sg[0:icw, :], pg[0:icw, :], AF.Silu)
                    nc.vector.tensor_mul(
                        act_sb[0:icw, ic, n * QW:(n + 1) * QW],
                        sg[0:icw, :], pu[0:icw, :])
            # down proj: nd-outer so ReduceScatter of first half overlaps
            for nd in range(NQ):
                dstpart = mlpA if nd < 2 else mlpB
                dlo = (nd % 2) * QW
                for sc_ in range(S // P):
                    pt = mpp.tile([P, QW], F32, space="PSUM", name="m_pd")
                    for ic in range(NIC):
                        icw = _icw(ic)
                        nc.tensor.matmul(
                            pt[:], act_sb[0:icw, ic, sc_ * P:(sc_ + 1) * P],
                            dw_sb[0:icw, ic, nd * QW:(nd + 1) * QW],
                            start=(ic == 0), stop=(ic == NIC - 1))
                    msb = mp.tile([P, QW], BF16, name="m_sb")
                    nc.vector.tensor_copy(msb[:], pt[:])
                    nc.sync.dma_start(
                        out=dstpart[sc_ * P:(sc_ + 1) * P, dlo:dlo + QW],
                        in_=msb[:])
                if nd == 1:
                    nc.gpsimd.collective_compute(
                        "ReduceScatter", OP.add, replica_groups=RG,
                        ins=[mlpA[:]], outs=[mlp_rsA[:]])
            nc.gpsimd.collective_compute(
                "ReduceScatter", OP.add, replica_groups=RG,
                ins=[mlpB[:]], outs=[mlp_rsB[:]])
        esM.close()

        # ============ Final ============
        with tc.tile_pool(name="fin", bufs=2) as fp:
            for half in range(2):
                mta = fp.tile([P, D // 2], BF16, name="f_mta")
                mtb = fp.tile([P, D // 2], BF16, name="f_mtb")
                nc.sync.dma_start(out=mta[:],
                                  in_=mlp_rsA[half * P:(half + 1) * P, :])
                nc.sync.dma_start(out=mtb[:],
                                  in_=mlp_rsB[half * P:(half + 1) * P, :])
                x2 = fp.tile([P, D], F32, name="f_x2")
                nc.vector.tensor_add(x2[:, 0:D // 2], x1[half][:, 0:D // 2],
                                     mta[:])
                nc.vector.tensor_add(x2[:, D // 2:D], x1[half][:, D // 2:D],
                                     mtb[:])
                nc.sync.dma_start(out=x2_out[half * P:(half + 1) * P, :],
                                  in_=x2[:])
                dlt = fp.tile([P, D], F32, name="f_dlt")
                nc.vector.tensor_sub(dlt[:], x2[:], selh[half][:])
                upd = fp.tile([P, D], F32, name="f_upd")
                nc.vector.scalar_tensor_tensor(
                    upd[:], in0=dlt[:], scalar=gate_g[half][:, :1],
                    in1=selh[half][:], op0=OP.mult, op1=OP.add)
                nc.sync.dma_start(out=upd_out[half * P:(half + 1) * P, :],
                                  in_=upd[:])
                nc.sync.dma_start(out=selidx_out[half * P:(half + 1) * P, :],
                                  in_=own_rows[half][:])
            nc.vector.tensor_copy(dbg_t[:, 8:9], gate_g[0][:])
            nc.sync.dma_start(out=dbg[:], in_=dbg_t[:])


def _rmsnorm_bf(nc, pool, x, out_bf, sq_scr, epst):
    """out_bf = bf16(x * rsqrt(mean(x^2)+eps)), x f32 [128, D]."""
    ssq = pool.tile([P, 1], F32, name="rn_ssq")
    nc.scalar.activation(sq_scr[:], x[:], AF.Square, accum_out=ssq[:])
    rt = pool.tile([P, 1], F32, name="rn_rt")
    nc.scalar.activation(rt[:], ssq[:], AF.Sqrt, scale=1.0 / D,
                         bias=epst[:, :1])
    rec = pool.tile([P, 1], F32, name="rn_rec")
    nc.vector.reciprocal(rec[:], rt[:])
    nc.scalar.activation(out_bf[:], x[:], AF.Copy, scale=rec[:, :1])


def _row_select_bcast(nc, pool, src_all, col_b, out_bcast):
    """out = broadcast(src_all row-block b), b in {0,1} from col_b."""
    r0 = pool.tile([1, T], F32, name="rs_r0")
    r1 = pool.tile([1, T], F32, name="rs_r1")
    v = src_all.rearrange("(a t) one -> a (t one)", a=2)
    nc.sync.dma_start(out=r0[:], in_=v[0:1, :])
    nc.sync.dma_start(out=r1[:], in_=v[1:2, :])
    b0 = pool.tile([P, T], F32, name="rs_b0")
    b1 = pool.tile([P, T], F32, name="rs_b1")
    nc.gpsimd.partition_broadcast(b0[:], r0[:])
    nc.gpsimd.partition_broadcast(b1[:], r1[:])
    df = pool.tile([P, T], F32, name="rs_df")
    nc.vector.tensor_sub(df[:], b1[:], b0[:])
    nc.vector.scalar_tensor_tensor(out_bcast[:], in0=df[:], scalar=col_b,
                                   in1=b0[:], op0=OP.mult, op1=OP.add)


# =====================================================================
# Host side
# =====================================================================
def kernel(**inputs):
    hs = np.asarray(inputs["hidden_states"], np.float32)
    qw = np.asarray(inputs["q_w"], np.float32)
    kw = np.asarray(inputs["k_w"], np.float32)
    vw = np.asarray(inputs["v_w"], np.float32)
    ow = np.asarray(inputs["o_w"], np.float32)
    bcu = float(np.asarray(inputs["beta_cu"]))
    bce = float(np.asarray(inputs["beta_ce"]))
    ceo = float(np.asarray(inputs["ce_off"]))

    hs_f = np.ascontiguousarray(hs.reshape(BT, D))
    orig_f = np.asarray(inputs["original"], np.float32).reshape(BT, D)
    post_f = np.asarray(inputs["posterior"], np.float32).reshape(BT, D)
    prior_f = np.asarray(inputs["prior"], np.float32).reshape(BT, D)
    cos_f = np.asarray(inputs["cos"], np.float32).reshape(BT, HD)
    sin_f = np.asarray(inputs["sin"], np.float32).reshape(BT, HD)
    sinm = sin_f.copy()
    sinm[:, : HD // 2] = -sinm[:, : HD // 2]
    cs_cat = np.ascontiguousarray(
        np.concatenate([cos_f, sinm], axis=1)).astype(BF16_NP)

    gw = np.asarray(inputs["gate_w"], np.float32)
    uw = np.asarray(inputs["up_w"], np.float32)
    dw = np.asarray(inputs["down_w"], np.float32)

    in_maps = []
    for c in range(NC):
        sl = slice(c * TOKS, (c + 1) * TOKS)
        hd_sl = slice(c * HPC * HD, (c + 1) * HPC * HD)
        ic_sl = slice(c * ICOL, (c + 1) * ICOL)
        b = c // 4
        cconst = np.array([[bcu, bce, bce * ceo, c * SB, 0.0,
                            0.0, (c % 4) * TOKS, b]], np.float32)
        in_maps.append({
            "orig_s": np.ascontiguousarray(orig_f[sl]),
            "post_s": np.ascontiguousarray(post_f[sl]),
            "prior_s": np.ascontiguousarray(prior_f[sl]),
            "hidden": hs_f,
            "cs_cat": cs_cat,
            "qw_s": np.ascontiguousarray(qw[:, hd_sl]).astype(BF16_NP),
            "kw_s": np.ascontiguousarray(kw[:, hd_sl]).astype(BF16_NP),
            "vw_s": np.ascontiguousarray(vw[:, hd_sl]).astype(BF16_NP),
            "ow_s": np.ascontiguousarray(ow[hd_sl, :]).astype(BF16_NP),
            "ln1w": np.asarray(inputs["ln1_w"], np.float32).reshape(-1, 1),
            "ln2w": np.asarray(inputs["ln2_w"], np.float32).reshape(-1, 1),
            "gatew_s": np.ascontiguousarray(gw[:, ic_sl]).astype(BF16_NP),
            "upw_s": np.ascontiguousarray(uw[:, ic_sl]).astype(BF16_NP),
            "downw_s": np.ascontiguousarray(dw[ic_sl, :]).astype(BF16_NP),
            "cconst": cconst,
        })

    global _last_in_maps
    _last_in_maps = in_maps
    import os
    ph = os.environ.get("KPHASES", "full")
    if ph not in _NC_CACHE:
        _NC_CACHE[ph] = build(phases=ph)
    nc = _NC_CACHE[ph]
    res = run_bass_kernel_spmd(nc, in_maps, core_ids=list(range(NC)))

    global _last_results
    _last_results = [res.results[c] for c in range(NC)]
    out = hs_f.copy()
    for c in range(NC):
        idx = res.results[c]["selidx_out"][:, 0]
        out[idx] = res.results[c]["upd_out"]
    return out.reshape(B, T, D)


if __name__ == "__main__":
    import reference
    inp = {k: np.asarray(v) for k, v in reference.setup_inputs().items()}
    got = kernel(**inp)
    want = np.asarray(reference.reference(**reference.setup_inputs()))
    err = np.abs(got - want).max() / np.abs(want).max()
    print("rel err:", err)


# revision 12
# speedup vs baseline: 1.9909x; 1.0108x over previous
"""Trainium2 Bass kernel for nn_DTFDynamicLayer (dynamic-token transformer
layer), SPMD across 8 NeuronCores — optimized v2.

kernel(**inputs) takes FULL unsharded numpy inputs (keys as in setup_inputs)
and returns the FULL [B,T,D] output. Sharding strategy:
  - router (scores/rank): token-sharded (512 tokens/core) + 2 tiny AllGathers;
    slot positions computed locally via prefix-scan (no 3rd AllGather)
  - packed sequence S=2048; attention is HEAD-parallel: each core computes
    Q/K/V and full causal attention for its 2 heads over all S positions,
    then a partial O-projection combined with ReduceScatter
  - MLP tensor-parallel over intermediate dim (704/core), partials combined
    with ReduceScatter
  - all matmuls in bf16 (f32 PSUM accumulation); router stays f32
"""
from contextlib import ExitStack

import numpy as np
import ml_dtypes

import concourse.bass as bass
import concourse.mybir as mybir
import concourse.tile as tile
from concourse import bacc
from concourse.bass_utils import run_bass_kernel_spmd
from concourse.masks import make_identity

B, T, D = 2, 2048, 2048
H, HD = 16, 128
I = 5632
EPS = 1e-6
NC = 8
BT = B * T
TOKS = BT // NC          # 512 router tokens per core
K = T // 2               # 1024 selected per batch row
S = B * K                # 2048 packed tokens
SB = S // NC             # 256 packed slots per core
HPC = H // NC            # 2 heads per core
ICOL = I // NC           # 704
DC = D // 128            # 16
NIC = (ICOL + 127) // 128  # 6 intermediate chunks (5x128 + 64)
SCALE = 1.0 / float(np.sqrt(HD))

F32 = mybir.dt.float32
BF16 = mybir.dt.bfloat16
I32 = mybir.dt.int32
AF = mybir.ActivationFunctionType
OP = mybir.AluOpType
P = 128
NQ = 4                  # 512-wide column chunks of S
QW = S // NQ            # 512
BF16_NP = ml_dtypes.bfloat16

_NC_CACHE = {}


def _icw(ic):
    return min(P, ICOL - ic * P)


def build(phases="full"):
    nc = bacc.Bacc(None, target_bir_lowering=False)
    _build(nc, phases)
    nc.finalize()
    return nc


def _build(nc, phases):
    dp = nc.declare_dram_parameter
    orig_s = dp("orig_s", [TOKS, D], F32, isOutput=False)
    post_s = dp("post_s", [TOKS, D], F32, isOutput=False)
    prior_s = dp("prior_s", [TOKS, D], F32, isOutput=False)
    hidden = dp("hidden", [BT, D], F32, isOutput=False)
    cs_cat = dp("cs_cat", [BT, 2 * HD], BF16, isOutput=False)  # [cos | sinm]
    qw_s = dp("qw_s", [D, HPC * HD], BF16, isOutput=False)
    kw_s = dp("kw_s", [D, HPC * HD], BF16, isOutput=False)
    vw_s = dp("vw_s", [D, HPC * HD], BF16, isOutput=False)
    ow_s = dp("ow_s", [HPC * HD, D], BF16, isOutput=False)
    ln1w = dp("ln1w", [D, 1], F32, isOutput=False)
    ln2w = dp("ln2w", [D, 1], F32, isOutput=False)
    gatew_s = dp("gatew_s", [D, ICOL], BF16, isOutput=False)
    upw_s = dp("upw_s", [D, ICOL], BF16, isOutput=False)
    downw_s = dp("downw_s", [ICOL, D], BF16, isOutput=False)
    # cconst: [beta_cu, beta_ce, beta_ce*ce_off, i0(=c*SB), 0, 0,
    #          i0row(=(c%4)*TOKS), b(=c//4)]
    cconst = dp("cconst", [1, 8], F32, isOutput=False)

    upd_out = dp("upd_out", [SB, D], F32, isOutput=True)
    x2_out = dp("x2_out", [SB, D], F32, isOutput=True)
    selidx_out = dp("selidx_out", [SB, 1], I32, isOutput=True)
    dbg = dp("dbg", [P, 16], F32, isOutput=True)

    RG = [list(range(NC))]

    with tile.TileContext(nc) as tc, ExitStack() as es:
        # -------- DRAM internals --------
        dr = es.enter_context(tc.tile_pool(name="dram", bufs=1, space="DRAM"))

        def dtile(name, shape, dtype=F32, shared=False):
            return dr.tile(shape, dtype, name=name,
                           addr_space="Shared" if shared else "Local")

        warm_in = dtile("warm_in", [1, 8])
        warm_all = dtile("warm_all", [NC, 8], shared=True)
        sc_in = dtile("sc_in", [TOKS, 1])
        sc_all = dtile("sc_all", [BT, 1], shared=True)
        mk_in = dtile("mk_in", [TOKS, 1])
        mk_all = dtile("mk_all", [BT, 1], shared=True)
        ps_d = dtile("ps_d", [BT, 1])
        selidx_d = dtile("selidx_d", [S + P, 1], I32)
        h1t_inA = dtile("h1t_inA", [D // 2, SB], BF16)
        h1t_inB = dtile("h1t_inB", [D // 2, SB], BF16)
        h1t_allA = dtile("h1t_allA", [NC * D // 2, SB], BF16, shared=True)
        h1t_allB = dtile("h1t_allB", [NC * D // 2, SB], BF16, shared=True)
        cs_own = dtile("cs_own", [SB, 2 * HD], BF16)
        cs_all = dtile("cs_all", [S, 2 * HD], BF16, shared=True)
        opart4 = [dtile(f"opart{i}", [S, QW], BF16) for i in range(NQ)]
        o_rs4 = [dtile(f"o_rs{i}", [SB, QW], BF16) for i in range(NQ)]
        h2t_inA = dtile("h2t_inA", [D // 2, SB], BF16)
        h2t_inB = dtile("h2t_inB", [D // 2, SB], BF16)
        h2t_allA = dtile("h2t_allA", [NC * D // 2, SB], BF16, shared=True)
        h2t_allB = dtile("h2t_allB", [NC * D // 2, SB], BF16, shared=True)
        mlp4 = [dtile(f"mlp{i}", [S, QW], BF16) for i in range(NQ)]
        mlp_rs4 = [dtile(f"mlp_rs{i}", [SB, QW], BF16) for i in range(NQ)]

        # -------- persistent SBUF --------
        pers = es.enter_context(tc.tile_pool(name="pers", bufs=1))
        ident_bf = pers.tile([P, P], BF16)
        make_identity(nc, ident_bf[:])
        cc_sb = pers.tile([1, 8], F32)
        nc.sync.dma_start(out=cc_sb[:], in_=cconst[:])
        ccb = pers.tile([P, 8], F32)
        nc.gpsimd.partition_broadcast(ccb[:], cc_sb[:])
        col_bcu = ccb[:, 0:1]
        col_bce = ccb[:, 1:2]
        col_ceo = ccb[:, 2:3]
        col_i0 = ccb[:, 3:4]
        col_i0row = ccb[:, 6:7]
        col_b = ccb[:, 7:8]
        ones_bf = pers.tile([P, 1], BF16)
        nc.vector.memset(ones_bf[:], 1.0)
        epst = pers.tile([P, 1], F32)
        nc.vector.memset(epst[:], EPS)
        lnw_cols = pers.tile([P, 2 * DC], F32)  # [:, 0:16]=ln1, [:,16:32]=ln2
        nc.sync.dma_start(out=lnw_cols[:, 0:DC],
                          in_=ln1w.rearrange("(d p) one -> p d one", p=P))
        nc.sync.dma_start(out=lnw_cols[:, DC:2 * DC],
                          in_=ln2w.rearrange("(d p) one -> p d one", p=P))
        # causal masks for diagonal 128x512 chunks: keep when
        # (f - p - off) >= 0, off = (jc - 4*qb)*128
        att_mask = pers.tile([P, 4, QW], BF16)
        for r in range(4):
            nc.gpsimd.memset(att_mask[:, r, :], 1.0)
            nc.gpsimd.affine_select(
                out=att_mask[:, r, :], in_=att_mask[:, r, :],
                compare_op=OP.is_ge, fill=0.0, base=-r * P,
                pattern=[[1, QW]], channel_multiplier=-1)
        # strict-upper 32x32 (k<c) with cross-batch-row block zeroed
        tri32 = pers.tile([32, 32], F32)
        nc.gpsimd.memset(tri32[:], 1.0)
        nc.gpsimd.affine_select(out=tri32[:], in_=tri32[:],
                                compare_op=OP.is_gt, fill=0.0, base=0,
                                pattern=[[1, 32]], channel_multiplier=-1)
        nc.vector.memset(tri32[0:16, 16:32], 0.0)
        dbg_t = pers.tile([P, 16], F32)
        nc.vector.memset(dbg_t[:], 0.0)

        s_cols = [pers.tile([P, 1], F32, name=f"s_col{t}") for t in range(4)]
        m_cols = [pers.tile([P, 1], F32, name=f"m_col{t}") for t in range(4)]

        # warm up the collective rings with a tiny AllGather ASAP
        wt = pers.tile([1, 8], F32)
        nc.vector.memset(wt[:], 1.0)
        nc.sync.dma_start(out=warm_in[:], in_=wt[:])
        nc.gpsimd.collective_compute("AllGather", OP.bypass, replica_groups=RG,
                                     ins=[warm_in[:]], outs=[warm_all[:]])

        # precompute rank-phase jlt tiles (independent of scores)
        esR = ExitStack()
        rpre = esR.enter_context(tc.tile_pool(name="rankpre", bufs=1))
        iota_jmp = rpre.tile([P, T], F32, name="iota_jmp")  # value = j - p
        _it2 = rpre.tile([P, T], I32, name="iota_jmp_i")
        nc.gpsimd.iota(_it2[:], pattern=[[1, T]], base=0,
                       channel_multiplier=-1)
        nc.vector.tensor_copy(iota_jmp[:], _it2[:])
        jlt4 = rpre.tile([P, 4, T], F32, name="jlt4")
        for t in range(4):
            rhs = rpre.tile([P, 1], F32, name="jl_rhs")
            nc.vector.tensor_scalar(rhs[:], col_i0row, float(t * P - 1),
                                    None, op0=OP.add)
            nc.vector.tensor_scalar(jlt4[:, t, :], iota_jmp[:], rhs[:, :1],
                                    None, op0=OP.is_le)

        # ============ Phase R1: scores for own 512 tokens ============
        with tc.tile_pool(name="router", bufs=2) as rp:
            for t in range(4):
                cu = rp.tile([P, 1], F32, name="cu")
                ce = rp.tile([P, 1], F32, name="ce")
                for (a_ap, b_ap, dst) in ((orig_s, post_s, cu),
                                          (post_s, prior_s, ce)):
                    at = rp.tile([P, D], F32, name="r_at")
                    bt = rp.tile([P, D], F32, name="r_bt")
                    nc.sync.dma_start(out=at[:], in_=a_ap[t * P:(t + 1) * P, :])
                    nc.sync.dma_start(out=bt[:], in_=b_ap[t * P:(t + 1) * P, :])
                    df = rp.tile([P, D], F32, name="r_df")
                    nc.vector.tensor_sub(df[:], at[:], bt[:])
                    sq = rp.tile([P, D], F32, name="r_sq")
                    ssq = rp.tile([P, 1], F32, name="r_ssq")
                    nc.scalar.activation(sq[:], df[:], AF.Square,
                                         accum_out=ssq[:])
                    nc.scalar.activation(dst[:], ssq[:], AF.Sqrt)
                t1 = rp.tile([P, 1], F32, name="r_t1")
                nc.vector.tensor_scalar(t1[:], cu[:], col_bcu, None,
                                        op0=OP.mult)
                nc.vector.scalar_tensor_tensor(
                    s_cols[t][:], in0=ce[:], scalar=col_bce, in1=t1[:],
                    op0=OP.mult, op1=OP.add)
                nc.vector.tensor_scalar(s_cols[t][:], s_cols[t][:], col_ceo,
                                        None, op0=OP.add)
            sc_flat = rp.tile([P, 4], F32, name="scflat")
            for t in range(4):
                nc.vector.tensor_copy(sc_flat[:, t:t + 1], s_cols[t][:])
            nc.sync.dma_start(
                out=sc_in.rearrange("(t p) one -> p t one", p=P),
                in_=sc_flat[:])
        nc.gpsimd.collective_compute("AllGather", OP.bypass, replica_groups=RG,
                                     ins=[sc_in[:]], outs=[sc_all[:]])

        # ============ Phase R2: rank -> mask for own tokens ============
        # rank_i = #{j: s_j>s_i} + #{j<i: s_j==s_i}; mask = rank <= K-1
        # <=> acc = sum(le) - sum(eq*jlt) >= T-K+1
        with tc.tile_pool(name="rank1", bufs=1) as rp1, \
             tc.tile_pool(name="rank", bufs=2) as rp:
            sbr = rp1.tile([P, T], F32, name="sbr")
            _row_select_bcast(nc, rp1, sc_all, col_b, sbr)
            for t in range(4):
                jlt = jlt4[:, t, :]
                le = rp.tile([P, T], F32, name="k_le")
                nc.vector.tensor_scalar(le[:], sbr[:], s_cols[t][:, :1], None,
                                        op0=OP.is_le)
                eq = rp.tile([P, T], F32, name="k_eq")
                nc.vector.tensor_scalar(eq[:], sbr[:], s_cols[t][:, :1], None,
                                        op0=OP.is_equal)
                eqlt = rp.tile([P, T], F32, name="k_eqlt")
                nc.vector.tensor_mul(eqlt[:], eq[:], jlt)
                dif = rp.tile([P, T], F32, name="k_dif")
                nc.vector.tensor_sub(dif[:], le[:], eqlt[:])
                acc = rp.tile([P, 1], F32, name="k_acc")
                nc.vector.tensor_reduce(acc[:], dif[:],
                                        axis=mybir.AxisListType.X, op=OP.add)
                nacc = rp.tile([P, 1], F32, name="k_nacc")
                nc.vector.tensor_scalar_mul(nacc[:], acc[:], -1.0)
                nc.vector.tensor_scalar(m_cols[t][:], nacc[:],
                                        float(-(T - K + 1)), None,
                                        op0=OP.is_le)
            mflat = rp.tile([P, 4], F32, name="mflat")
            for t in range(4):
                nc.vector.tensor_copy(mflat[:, t:t + 1], m_cols[t][:])
            nc.sync.dma_start(
                out=mk_in.rearrange("(t p) one -> p t one", p=P), in_=mflat[:])
        nc.gpsimd.collective_compute("AllGather", OP.bypass, replica_groups=RG,
                                     ins=[mk_in[:]], outs=[mk_all[:]])
        esR.close()

        # ============ Phase R3: positions for ALL tokens (local) ============
        # layout [32 chunks (partition), 128 tokens (free)]; exclusive prefix
        # within chunk by shift+doubling; chunk offsets via tri32 matmul.
        with tc.tile_pool(name="pos", bufs=1) as pp, \
             tc.tile_pool(name="posp", bufs=1, space="PSUM") as ppp:
            mk_c = pp.tile([32, P], F32, name="mk_c")
            nc.sync.dma_start(out=mk_c[:],
                              in_=mk_all.rearrange("(c q) one -> c (q one)",
                                                   c=32))
            exA = pp.tile([32, P], F32, name="exA")
            exB = pp.tile([32, P], F32, name="exB")
            nc.vector.memset(exA[:, 0:1], 0.0)
            nc.vector.tensor_copy(exA[:, 1:P], mk_c[:, 0:P - 1])
            cur, nxt = exA, exB
            k = 1
            while k < P:
                nc.vector.tensor_copy(nxt[:, 0:k], cur[:, 0:k])
                nc.vector.tensor_add(nxt[:, k:P], cur[:, k:P], cur[:, 0:P - k])
                cur, nxt = nxt, cur
                k *= 2
            tot_col = pp.tile([32, 1], F32, name="tot_col")
            nc.vector.tensor_add(tot_col[:], cur[:, P - 1:P],
                                 mk_c[:, P - 1:P])
            ps_off = ppp.tile([32, 1], F32, space="PSUM", name="ps_off")
            nc.tensor.matmul(ps_off[:], tri32[:], tot_col[:], start=True,
                             stop=True)
            off_sb = pp.tile([32, 1], F32, name="off_sb")
            nc.vector.tensor_copy(off_sb[:], ps_off[:])
            pos_c = pp.tile([32, P], F32, name="pos_c")
            nc.vector.tensor_scalar(pos_c[:], cur[:], off_sb[:, :1], None,
                                    op0=OP.add)
            nc.sync.dma_start(
                out=ps_d.rearrange("(c q) one -> c (q one)", c=32),
                in_=pos_c[:])

        # ============ Phase SCT: slot -> flat row map ============
        with tc.tile_pool(name="scat", bufs=4) as sp:
            zt = sp.tile([P, (S + P) // P], I32, name="sc_zero")
            nc.vector.memset(zt[:], 0)
            nc.sync.dma_start(
                out=selidx_d.rearrange("(t p) one -> p t one", p=P), in_=zt[:])
            mk_t = sp.tile([P, BT // P], F32, name="mk_t")
            ps_t = sp.tile([P, BT // P], F32, name="ps_t")
            nc.sync.dma_start(out=mk_t[:],
                              in_=mk_all.rearrange("(t p) one -> p t one", p=P))
            nc.sync.dma_start(out=ps_t[:],
                              in_=ps_d.rearrange("(t p) one -> p t one", p=P))
            dump_i = sp.tile([P, 1], I32, name="sc_dumpi")
            nc.gpsimd.iota(dump_i[:], pattern=[[0, 1]], base=S,
                           channel_multiplier=1)
            dump_f = sp.tile([P, 1], F32, name="sc_dumpf")
            nc.vector.tensor_copy(dump_f[:], dump_i[:])
            for t in range(BT // P):
                b = (t * P) // T
                # slot' = m*(pos + b*K - (S+p)) + (S+p)
                t1 = sp.tile([P, 1], F32, name="sc_t1")
                nc.vector.tensor_scalar(t1[:], ps_t[:, t:t + 1],
                                        float(b * K), None, op0=OP.add)
                nc.vector.tensor_sub(t1[:], t1[:], dump_f[:])
                t2 = sp.tile([P, 1], F32, name="sc_t2")
                nc.vector.tensor_mul(t2[:], t1[:], mk_t[:, t:t + 1])
                nc.vector.tensor_add(t2[:], t2[:], dump_f[:])
                off_i = sp.tile([P, 1], I32, name="sc_off")
                nc.vector.tensor_copy(off_i[:], t2[:])
                val_i = sp.tile([P, 1], I32, name="sc_val")
                nc.gpsimd.iota(val_i[:], pattern=[[0, 1]], base=t * P,
                               channel_multiplier=1)
                nc.gpsimd.indirect_dma_start(
                    out=selidx_d[:],
                    out_offset=bass.IndirectOffsetOnAxis(ap=off_i[:, :1],
                                                         axis=0),
                    in_=val_i[:], in_offset=None)

        # ============ Phase G: gathers ============
        gpL = es.enter_context(tc.tile_pool(name="gpL", bufs=1))   # long-lived
        own_rows = []
        selh = []
        gate_g = []
        myslot = gpL.tile([P, 2], I32)
        _si = gpL.tile([P, 2], I32)
        _slotf = gpL.tile([P, 2], F32)
        for half in range(2):
            nc.gpsimd.iota(_si[:, half:half + 1], pattern=[[0, 1]],
                           base=half * P, channel_multiplier=1)
        nc.vector.tensor_copy(_slotf[:], _si[:])
        for half in range(2):
            nc.vector.tensor_scalar(_slotf[:, half:half + 1],
                                    _slotf[:, half:half + 1], col_i0, None,
                                    op0=OP.add)
        nc.vector.tensor_copy(myslot[:], _slotf[:])
        for half in range(2):
            orow = gpL.tile([P, 1], I32, name=f"orow{half}")
            nc.gpsimd.indirect_dma_start(
                out=orow[:], out_offset=None, in_=selidx_d[:],
                in_offset=bass.IndirectOffsetOnAxis(
                    ap=myslot[:, half:half + 1], axis=0))
            own_rows.append(orow)
        for half in range(2):
            sh = gpL.tile([P, D], F32, name=f"selh{half}")
            nc.gpsimd.indirect_dma_start(
                out=sh[:], out_offset=None, in_=hidden[:],
                in_offset=bass.IndirectOffsetOnAxis(
                    ap=own_rows[half][:, :1], axis=0),
                bounds_check=BT - 1, oob_is_err=False)
            selh.append(sh)
        # gather own slots' cos|sinm rows; AllGather to all cores
        for half in range(2):
            csh = gpL.tile([P, 2 * HD], BF16, name=f"csh{half}")
            nc.gpsimd.indirect_dma_start(
                out=csh[:], out_offset=None, in_=cs_cat[:],
                in_offset=bass.IndirectOffsetOnAxis(
                    ap=own_rows[half][:, :1], axis=0))
            nc.sync.dma_start(out=cs_own[half * P:(half + 1) * P, :],
                              in_=csh[:])
        nc.gpsimd.collective_compute("AllGather", OP.bypass, replica_groups=RG,
                                     ins=[cs_own[:]], outs=[cs_all[:]])
        for half in range(2):
            ssc = gpL.tile([P, 1], F32, name=f"ssc{half}")
            nc.gpsimd.indirect_dma_start(
                out=ssc[:], out_offset=None, in_=sc_all[:],
                in_offset=bass.IndirectOffsetOnAxis(
                    ap=own_rows[half][:, :1], axis=0))
            gg = gpL.tile([P, 1], F32, name=f"gate{half}")
            nc.scalar.activation(gg[:], ssc[:], AF.Sigmoid)
            gate_g.append(gg)
        x1 = [gpL.tile([P, D], F32, name=f"x1_{i}") for i in range(2)]

        if phases != "full":
            with tc.tile_pool(name="rfin", bufs=2) as fp:
                for half in range(2):
                    nc.sync.dma_start(
                        out=upd_out[half * P:(half + 1) * P, :],
                        in_=selh[half][:])
                    nc.sync.dma_start(
                        out=x2_out[half * P:(half + 1) * P, :],
                        in_=selh[half][:])
                    nc.sync.dma_start(out=selidx_out[half * P:(half + 1) * P, :],
                                      in_=own_rows[half][:])
                nc.vector.tensor_copy(dbg_t[:, 5:6], gate_g[0][:])
                nc.sync.dma_start(out=dbg[:], in_=dbg_t[:])
            return

        # ============ Phase N1: h1 = rmsnorm(selh)*ln1; AG feature-major ====
        with tc.tile_pool(name="n1", bufs=2) as np_, \
             tc.tile_pool(name="n1p", bufs=4, space="PSUM") as npp:
            h1T_own = np_.tile([P, DC, SB], BF16, name="h1T_own")
            sq_scr = np_.tile([P, D], F32, name="sq_scr1")
            for half in range(2):
                h1b = np_.tile([P, D], BF16, name="h1b")
                _rmsnorm_bf(nc, np_, selh[half], h1b, sq_scr, epst)
                for d in range(DC):
                    pt = npp.tile([P, P], BF16, space="PSUM", name="n1_tp")
                    nc.tensor.transpose(pt[:], h1b[:, d * P:(d + 1) * P],
                                        ident_bf[:])
                    nc.vector.tensor_scalar(
                        h1T_own[:, d, half * P:(half + 1) * P], pt[:],
                        lnw_cols[:, d:d + 1], None, op0=OP.mult)
            for d in range(DC):
                dst = h1t_inA if d < 8 else h1t_inB
                dd = d % 8
                nc.sync.dma_start(out=dst[dd * P:(dd + 1) * P, :],
                                  in_=h1T_own[:, d, :])
        nc.gpsimd.collective_compute("AllGather", OP.bypass, replica_groups=RG,
                                     ins=[h1t_inA[:]], outs=[h1t_allA[:]])
        nc.gpsimd.collective_compute("AllGather", OP.bypass, replica_groups=RG,
                                     ins=[h1t_inB[:]], outs=[h1t_allB[:]])

        # attention-lived pool (qh/kh/vtok/o_fm survive into OPROJ)
        esA = ExitStack()
        gpA = esA.enter_context(tc.tile_pool(name="gpA", bufs=1))
        qh = [gpA.tile([P, S], BF16, name=f"qh{h}") for h in range(HPC)]
        kh = [gpA.tile([P, S], BF16, name=f"kh{h}") for h in range(HPC)]
        vtok = [gpA.tile([P, S // P, HD], BF16, name=f"vtok{h}")
                for h in range(HPC)]
        o_fm = [gpA.tile([P, S], BF16, name=f"ofm{h}") for h in range(HPC)]
        ow_sb = gpA.tile([P, HPC, D], BF16, name="ow_sb")
        for h in range(HPC):
            nc.sync.dma_start(out=ow_sb[:, h, :],
                              in_=ow_s[h * P:(h + 1) * P, :])

        # ============ Phase QKV (own 2 heads, full S) ============
        esQ = ExitStack()
        gpQ = esQ.enter_context(tc.tile_pool(name="gpQ", bufs=1))
        h1T = gpQ.tile([P, DC, S], BF16, name="h1T")
        for d in range(DC):
            src = h1t_allA if d < 8 else h1t_allB
            dd = d % 8
            eng = nc.sync if d % 2 == 0 else nc.scalar
            for j in range(NC):
                eng.dma_start(
                    out=h1T[:, d, j * SB:(j + 1) * SB],
                    in_=src[j * (D // 2) + dd * P:j * (D // 2) + (dd + 1) * P,
                            :])
        qw_sb = gpQ.tile([P, DC, HPC * HD], BF16, name="qw_sb")
        kw_sb = gpQ.tile([P, DC, HPC * HD], BF16, name="kw_sb")
        vw_sb = gpQ.tile([P, DC, HPC * HD], BF16, name="vw_sb")
        for (wsb, wsrc) in ((qw_sb, qw_s), (kw_sb, kw_s), (vw_sb, vw_s)):
            for d in range(DC):
                nc.sync.dma_start(out=wsb[:, d, :],
                                  in_=wsrc[d * P:(d + 1) * P, :])
        # cos|sinm for all S slots (from AG), transposed to feature-major
        cosT = gpQ.tile([P, S], BF16, name="cosT")
        sinmT = gpQ.tile([P, S], BF16, name="sinmT")
        with tc.tile_pool(name="csg", bufs=3) as cp, \
             tc.tile_pool(name="csgp", bufs=4, space="PSUM") as cpp:
            for sc_ in range(S // P):
                csg = cp.tile([P, 2 * HD], BF16, name="csg")
                nc.sync.dma_start(out=csg[:],
                                  in_=cs_all[sc_ * P:(sc_ + 1) * P, :])
                for (lo, dstT) in ((0, cosT), (HD, sinmT)):
                    pt = cpp.tile([P, P], BF16, space="PSUM", name="cs_p")
                    nc.tensor.transpose(pt[:], csg[:, lo:lo + HD], ident_bf[:])
                    nc.vector.tensor_copy(dstT[:, sc_ * P:(sc_ + 1) * P],
                                          pt[:])

        with tc.tile_pool(name="qkv", bufs=3) as qp, \
             tc.tile_pool(name="qkvp", bufs=2, space="PSUM") as qpp:
            for h in range(HPC):
                for (wsb, dsth) in ((qw_sb, qh), (kw_sb, kh)):
                    for n in range(NQ):
                        pt = qpp.tile([P, QW], F32, space="PSUM", name="qk_ps")
                        for d in range(DC):
                            nc.tensor.matmul(
                                pt[:], wsb[:, d, h * HD:(h + 1) * HD],
                                h1T[:, d, n * QW:(n + 1) * QW],
                                start=(d == 0), stop=(d == DC - 1))
                        # rope: out = pt*cos + rot(pt)*sinm
                        rot = qp.tile([P, QW], F32, name="rp_rot")
                        nc.vector.tensor_copy(rot[0:64, :], pt[64:P, :])
                        nc.vector.tensor_copy(rot[64:P, :], pt[0:64, :])
                        t1 = qp.tile([P, QW], F32, name="rp_t1")
                        nc.vector.tensor_mul(
                            t1[:], pt[:], cosT[:, n * QW:(n + 1) * QW])
                        t2 = qp.tile([P, QW], F32, name="rp_t2")
                        nc.vector.tensor_mul(
                            t2[:], rot[:], sinmT[:, n * QW:(n + 1) * QW])
                        nc.vector.tensor_add(
                            dsth[h][:, n * QW:(n + 1) * QW], t1[:], t2[:])
                for n in range(NQ):
                    pt = qpp.tile([P, QW], F32, space="PSUM", name="v_ps")
                    for d in range(DC):
                        nc.tensor.matmul(
                            pt[:], vw_sb[:, d, h * HD:(h + 1) * HD],
                            h1T[:, d, n * QW:(n + 1) * QW],
                            start=(d == 0), stop=(d == DC - 1))
                    vsb = qp.tile([P, QW], BF16, name="v_sb")
                    nc.vector.tensor_copy(vsb[:], pt[:])
                    for kk in range(NQ):
                        ptt = qpp.tile([P, P], BF16, space="PSUM", name="vt_ps")
                        nc.tensor.transpose(ptt[:], vsb[:, kk * P:(kk + 1) * P],
                                            ident_bf[:])
                        nc.vector.tensor_copy(vtok[h][:, n * NQ + kk, :],
                                              ptt[:])
        esQ.close()

        # ============ Phase ATT (own heads, causal, full S queries) ========
        with tc.tile_pool(name="att", bufs=4) as ap, \
             tc.tile_pool(name="attpa", bufs=3, space="PSUM") as apa, \
             tc.tile_pool(name="attpo", bufs=2, space="PSUM") as apo:
            for h in range(HPC):
                for qb in range(NQ):
                    jmax = 4 * (qb + 1)
                    po = apo.tile([P, QW], F32, space="PSUM", name="a_po")
                    psum = apo.tile([1, QW], F32, space="PSUM", name="a_ps")
                    for jc in range(jmax):
                        pa = apa.tile([P, QW], F32, space="PSUM", name="a_pa")
                        nc.tensor.matmul(pa[:], kh[h][:, jc * P:(jc + 1) * P],
                                         qh[h][:, qb * QW:(qb + 1) * QW],
                                         start=True, stop=True)
                        et = ap.tile([P, QW], BF16, name="a_et")
                        nc.scalar.activation(et[:], pa[:], AF.Exp, scale=SCALE)
                        if jc >= 4 * qb:
                            nc.vector.tensor_mul(et[:], et[:],
                                                 att_mask[:, jc - 4 * qb, :])
                        nc.tensor.matmul(psum[:], ones_bf[:], et[:],
                                         start=(jc == 0), stop=(jc == jmax - 1),
                                         skip_group_check=True)
                        nc.tensor.matmul(po[:], vtok[h][:, jc, :], et[:],
                                         start=(jc == 0), stop=(jc == jmax - 1),
                                         skip_group_check=True)
                    rec = ap.tile([1, QW], F32, name="a_rec")
                    nc.vector.reciprocal(rec[:], psum[:])
                    recb = ap.tile([P, QW], F32, name="a_recb")
                    nc.gpsimd.partition_broadcast(recb[:], rec[:])
                    nc.vector.tensor_mul(o_fm[h][:, qb * QW:(qb + 1) * QW],
                                         po[:], recb[:])

        # ============ Phase OPROJ: opart[s,d] = sum_h o_fm_h.T @ ow_h ======
        with tc.tile_pool(name="opj", bufs=3) as op_, \
             tc.tile_pool(name="opjp", bufs=3, space="PSUM") as opp:
            for nd in range(NQ):
                for qc in range(S // P):
                    pt = opp.tile([P, QW], F32, space="PSUM", name="o_ps")
                    for h in range(HPC):
                        nc.tensor.matmul(
                            pt[:], o_fm[h][:, qc * P:(qc + 1) * P],
                            ow_sb[:, h, nd * QW:(nd + 1) * QW],
                            start=(h == 0), stop=(h == HPC - 1))
                    osb = op_.tile([P, QW], BF16, name="o_sb")
                    nc.vector.tensor_copy(osb[:], pt[:])
                    nc.sync.dma_start(
                        out=opart4[nd][qc * P:(qc + 1) * P, :], in_=osb[:])
                nc.gpsimd.collective_compute(
                    "ReduceScatter", OP.add, replica_groups=RG,
                    ins=[opart4[nd][:]], outs=[o_rs4[nd][:]])
        esA.close()

        # MLP weights (loads overlap RS_o / N2 / AG_h2)
        esM = ExitStack()
        gpMw = esM.enter_context(tc.tile_pool(name="gpMw", bufs=1))
        gw_sb = gpMw.tile([P, DC, ICOL], BF16, name="gw_sb")
        uw_sb = gpMw.tile([P, DC, ICOL], BF16, name="uw_sb")
        dw_sb = gpMw.tile([P, NIC, D], BF16, name="dw_sb")
        for (wsb, wsrc) in ((gw_sb, gatew_s), (uw_sb, upw_s)):
            for d in range(DC):
                nc.scalar.dma_start(out=wsb[:, d, :],
                                    in_=wsrc[d * P:(d + 1) * P, :])
        for ic in range(NIC):
            icw = _icw(ic)
            nc.scalar.dma_start(out=dw_sb[0:icw, ic, :],
                                in_=downw_s[ic * P:ic * P + icw, :])

        # ============ Phase N2 + AG ============
        with tc.tile_pool(name="n2", bufs=2) as np2, \
             tc.tile_pool(name="n2p", bufs=4, space="PSUM") as npp2:
            h2T_own = np2.tile([P, DC, SB], BF16, name="h2T_own")
            sq_scr = np2.tile([P, D], F32, name="sq_scr2")
            for half in range(2):
                for nd in range(NQ):
                    ors = np2.tile([P, QW], BF16, name=f"ors{nd}")
                    nc.sync.dma_start(out=ors[:],
                                      in_=o_rs4[nd][half * P:(half + 1) * P, :])
                    nc.vector.tensor_add(
                        x1[half][:, nd * QW:(nd + 1) * QW],
                        selh[half][:, nd * QW:(nd + 1) * QW], ors[:])
                h2b = np2.tile([P, D], BF16, name="h2b")
                _rmsnorm_bf(nc, np2, x1[half], h2b, sq_scr, epst)
                for d in range(DC):
                    pt = npp2.tile([P, P], BF16, space="PSUM", name="n2_tp")
                    nc.tensor.transpose(pt[:], h2b[:, d * P:(d + 1) * P],
                                        ident_bf[:])
                    nc.vector.tensor_scalar(
                        h2T_own[:, d, half * P:(half + 1) * P], pt[:],
                        lnw_cols[:, DC + d:DC + d + 1], None, op0=OP.mult)
            for d in range(DC):
                dst = h2t_inA if d < 8 else h2t_inB
                dd = d % 8
                nc.sync.dma_start(out=dst[dd * P:(dd + 1) * P, :],
                                  in_=h2T_own[:, d, :])
        nc.gpsimd.collective_compute("AllGather", OP.bypass, replica_groups=RG,
                                     ins=[h2t_inA[:]], outs=[h2t_allA[:]])
        nc.gpsimd.collective_compute("AllGather", OP.bypass, replica_groups=RG,
                                     ins=[h2t_inB[:]], outs=[h2t_allB[:]])

        # ============ Phase MLP (TP over I) ============
        gpMa = esM.enter_context(tc.tile_pool(name="gpMa", bufs=1))
        h2T = gpMa.tile([P, DC, S], BF16, name="h2T")
        for d in range(DC):
            src = h2t_allA if d < 8 else h2t_allB
            dd = d % 8
            eng = nc.sync if d % 2 == 0 else nc.scalar
            for j in range(NC):
                eng.dma_start(
                    out=h2T[:, d, j * SB:(j + 1) * SB],
                    in_=src[j * (D // 2) + dd * P:j * (D // 2) + (dd + 1) * P,
                            :])
        act_sb = gpMa.tile([P, NIC, S], BF16, name="act_sb")
        with tc.tile_pool(name="mlp", bufs=3) as mp, \
             tc.tile_pool(name="mlpp", bufs=2, space="PSUM") as mpp:
            for n in range(NQ):
                for ic in range(NIC):
                    icw = _icw(ic)
                    pg = mpp.tile([P, QW], F32, space="PSUM", name="m_pg")
                    pu = mpp.tile([P, QW], F32, space="PSUM", name="m_pu")
                    for d in range(DC):
                        nc.tensor.matmul(pg[0:icw, :],
                                         gw_sb[:, d, ic * P:ic * P + icw],
                                         h2T[:, d, n * QW:(n + 1) * QW],
                                         start=(d == 0), stop=(d == DC - 1))
                    for d in range(DC):
                        nc.tensor.matmul(pu[0:icw, :],
                                         uw_sb[:, d, ic * P:ic * P + icw],
                                         h2T[:, d, n * QW:(n + 1) * QW],
                                         start=(d == 0), stop=(d == DC - 1))
                    sg = mp.tile([P, QW], BF16, name="m_sg")
                    nc.scalar.activation(sg[0:icw, :], pg[0:icw, :], AF.Silu)
                    nc.vector.tensor_mul(
                        act_sb[0:icw, ic, n * QW:(n + 1) * QW],
                        sg[0:icw, :], pu[0:icw, :])
            # down proj: nd-outer; ReduceScatter per nd overlaps next nd
            for nd in range(NQ):
                for sc_ in range(S // P):
                    pt = mpp.tile([P, QW], F32, space="PSUM", name="m_pd")
                    for ic in range(NIC):
                        icw = _icw(ic)
                        nc.tensor.matmul(
                            pt[:], act_sb[0:icw, ic, sc_ * P:(sc_ + 1) * P],
                            dw_sb[0:icw, ic, nd * QW:(nd + 1) * QW],
                            start=(ic == 0), stop=(ic == NIC - 1))
                    msb = mp.tile([P, QW], BF16, name="m_sb")
                    nc.vector.tensor_copy(msb[:], pt[:])
                    nc.sync.dma_start(
                        out=mlp4[nd][sc_ * P:(sc_ + 1) * P, :], in_=msb[:])
                nc.gpsimd.collective_compute(
                    "ReduceScatter", OP.add, replica_groups=RG,
                    ins=[mlp4[nd][:]], outs=[mlp_rs4[nd][:]])
        esM.close()

        # ============ Final ============
        with tc.tile_pool(name="fin", bufs=2) as fp:
            for half in range(2):
                x2 = fp.tile([P, D], F32, name="f_x2")
                for nd in range(NQ):
                    mt = fp.tile([P, QW], BF16, name=f"f_mt{nd}")
                    nc.sync.dma_start(
                        out=mt[:], in_=mlp_rs4[nd][half * P:(half + 1) * P, :])
                    nc.vector.tensor_add(x2[:, nd * QW:(nd + 1) * QW],
                                         x1[half][:, nd * QW:(nd + 1) * QW],
                                         mt[:])
                nc.sync.dma_start(out=x2_out[half * P:(half + 1) * P, :],
                                  in_=x2[:])
                dlt = fp.tile([P, D], F32, name="f_dlt")
                nc.vector.tensor_sub(dlt[:], x2[:], selh[half][:])
                upd = fp.tile([P, D], F32, name="f_upd")
                nc.vector.scalar_tensor_tensor(
                    upd[:], in0=dlt[:], scalar=gate_g[half][:, :1],
                    in1=selh[half][:], op0=OP.mult, op1=OP.add)
                nc.sync.dma_start(out=upd_out[half * P:(half + 1) * P, :],
                                  in_=upd[:])
                nc.sync.dma_start(out=selidx_out[half * P:(half + 1) * P, :],
                                  in_=own_rows[half][:])
            nc.vector.tensor_copy(dbg_t[:, 8:9], gate_g[0][:])
            nc.sync.dma_start(out=dbg[:], in_=dbg_t[:])


def _rmsnorm_bf(nc, pool, x, out_bf, sq_scr, epst):
    """out_bf = bf16(x * rsqrt(mean(x^2)+eps)), x f32 [128, D]."""
    ssq = pool.tile([P, 1], F32, name="rn_ssq")
    nc.scalar.activation(sq_scr[:], x[:], AF.Square, accum_out=ssq[:])
    rt = pool.tile([P, 1], F32, name="rn_rt")
    nc.scalar.activation(rt[:], ssq[:], AF.Sqrt, scale=1.0 / D,
                         bias=epst[:, :1])
    rec = pool.tile([P, 1], F32, name="rn_rec")
    nc.vector.reciprocal(rec[:], rt[:])
    nc.scalar.activation(out_bf[:], x[:], AF.Copy, scale=rec[:, :1])


def _row_select_bcast(nc, pool, src_all, col_b, out_bcast):
    """out = broadcast(src_all row-block b), b in {0,1} from col_b."""
    r0 = pool.tile([1, T], F32, name="rs_r0")
    r1 = pool.tile([1, T], F32, name="rs_r1")
    v = src_all.rearrange("(a t) one -> a (t one)", a=2)
    nc.sync.dma_start(out=r0[:], in_=v[0:1, :])
    nc.sync.dma_start(out=r1[:], in_=v[1:2, :])
    b0 = pool.tile([P, T], F32, name="rs_b0")
    b1 = pool.tile([P, T], F32, name="rs_b1")
    nc.gpsimd.partition_broadcast(b0[:], r0[:])
    nc.gpsimd.partition_broadcast(b1[:], r1[:])
    df = pool.tile([P, T], F32, name="rs_df")
    nc.vector.tensor_sub(df[:], b1[:], b0[:])
    nc.vector.scalar_tensor_tensor(out_bcast[:], in0=df[:], scalar=col_b,
                                   in1=b0[:], op0=OP.mult, op1=OP.add)


# =====================================================================
# Host side
# =====================================================================
def kernel(**inputs):
    hs = np.asarray(inputs["hidden_states"], np.float32)
    qw = np.asarray(inputs["q_w"], np.float32)
    kw = np.asarray(inputs["k_w"], np.float32)
    vw = np.asarray(inputs["v_w"], np.float32)
    ow = np.asarray(inputs["o_w"], np.float32)
    bcu = float(np.asarray(inputs["beta_cu"]))
    bce = float(np.asarray(inputs["beta_ce"]))
    ceo = float(np.asarray(inputs["ce_off"]))

    hs_f = np.ascontiguousarray(hs.reshape(BT, D))
    orig_f = np.asarray(inputs["original"], np.float32).reshape(BT, D)
    post_f = np.asarray(inputs["posterior"], np.float32).reshape(BT, D)
    prior_f = np.asarray(inputs["prior"], np.float32).reshape(BT, D)
    cos_f = np.asarray(inputs["cos"], np.float32).reshape(BT, HD)
    sin_f = np.asarray(inputs["sin"], np.float32).reshape(BT, HD)
    sinm = sin_f.copy()
    sinm[:, : HD // 2] = -sinm[:, : HD // 2]
    cs_cat = np.ascontiguousarray(
        np.concatenate([cos_f, sinm], axis=1)).astype(BF16_NP)

    gw = np.asarray(inputs["gate_w"], np.float32)
    uw = np.asarray(inputs["up_w"], np.float32)
    dw = np.asarray(inputs["down_w"], np.float32)

    in_maps = []
    for c in range(NC):
        sl = slice(c * TOKS, (c + 1) * TOKS)
        hd_sl = slice(c * HPC * HD, (c + 1) * HPC * HD)
        ic_sl = slice(c * ICOL, (c + 1) * ICOL)
        b = c // 4
        cconst = np.array([[bcu, bce, bce * ceo, c * SB, 0.0,
                            0.0, (c % 4) * TOKS, b]], np.float32)
        in_maps.append({
            "orig_s": np.ascontiguousarray(orig_f[sl]),
            "post_s": np.ascontiguousarray(post_f[sl]),
            "prior_s": np.ascontiguousarray(prior_f[sl]),
            "hidden": hs_f,
            "cs_cat": cs_cat,
            "qw_s": np.ascontiguousarray(qw[:, hd_sl]).astype(BF16_NP),
            "kw_s": np.ascontiguousarray(kw[:, hd_sl]).astype(BF16_NP),
            "vw_s": np.ascontiguousarray(vw[:, hd_sl]).astype(BF16_NP),
            "ow_s": np.ascontiguousarray(ow[hd_sl, :]).astype(BF16_NP),
            "ln1w": np.asarray(inputs["ln1_w"], np.float32).reshape(-1, 1),
            "ln2w": np.asarray(inputs["ln2_w"], np.float32).reshape(-1, 1),
            "gatew_s": np.ascontiguousarray(gw[:, ic_sl]).astype(BF16_NP),
            "upw_s": np.ascontiguousarray(uw[:, ic_sl]).astype(BF16_NP),
            "downw_s": np.ascontiguousarray(dw[ic_sl, :]).astype(BF16_NP),
            "cconst": cconst,
        })

    global _last_in_maps
    _last_in_maps = in_maps
    import os
    ph = os.environ.get("KPHASES", "full")
    if ph not in _NC_CACHE:
        _NC_CACHE[ph] = build(phases=ph)
    nc = _NC_CACHE[ph]
    res = run_bass_kernel_spmd(nc, in_maps, core_ids=list(range(NC)))

    global _last_results
    _last_results = [res.results[c] for c in range(NC)]
    out = hs_f.copy()
    for c in range(NC):
        idx = res.results[c]["selidx_out"][:, 0]
        out[idx] = res.results[c]["upd_out"]
    return out.reshape(B, T, D)


if __name__ == "__main__":
    import reference
    inp = {k: np.asarray(v) for k, v in reference.setup_inputs().items()}
    got = kernel(**inp)
    want = np.asarray(reference.reference(**reference.setup_inputs()))
    err = np.abs(got - want).max() / np.abs(want).max()
    print("rel err:", err)


# revision 14
# speedup vs baseline: 2.1697x; 1.0898x over previous
"""Trainium2 Bass kernel for nn_DTFDynamicLayer (dynamic-token transformer
layer), SPMD across 8 NeuronCores — optimized v4.

kernel(**inputs) takes FULL unsharded numpy inputs (keys as in setup_inputs)
and returns the FULL [B,T,D] output. Sharding strategy:
  - router (scores/rank): token-sharded (512 tokens/core) + 2 tiny AllGathers;
    slot positions via local prefix-scan; own packed rows found by direct
    position matching (no scatter/inverse-map round-trip)
  - packed sequence S=2048; attention is HEAD-parallel: each core computes
    Q/K/V and full causal attention for its 2 heads over all S positions,
    then a partial O-projection combined with ReduceScatter
  - MLP tensor-parallel over intermediate dim (704/core), partials combined
    with ReduceScatter
  - normed activations AllGathered token-major (2KB DMA lines), transposed
    to feature-major on the consumer side (pipelined with the matmuls)
  - all matmuls in bf16 (f32 PSUM accumulation); router stays f32
"""
from contextlib import ExitStack

import numpy as np
import ml_dtypes

import concourse.bass as bass
import concourse.mybir as mybir
import concourse.tile as tile
from concourse import bacc
from concourse.bass_utils import run_bass_kernel_spmd
from concourse.masks import make_identity

B, T, D = 2, 2048, 2048
H, HD = 16, 128
I = 5632
EPS = 1e-6
NC = 8
BT = B * T
TOKS = BT // NC          # 512 router tokens per core
K = T // 2               # 1024 selected per batch row
S = B * K                # 2048 packed tokens
SB = S // NC             # 256 packed slots per core
HPC = H // NC            # 2 heads per core
ICOL = I // NC           # 704
DC = D // 128            # 16
NIC = (ICOL + 127) // 128  # 6 intermediate chunks (5x128 + 64)
SCALE = 1.0 / float(np.sqrt(HD))

F32 = mybir.dt.float32
BF16 = mybir.dt.bfloat16
I32 = mybir.dt.int32
AF = mybir.ActivationFunctionType
OP = mybir.AluOpType
P = 128
NQ = 4                  # 512-wide column chunks of S
QW = S // NQ            # 512
HB = D // 2             # 1024 (feature half)
BF16_NP = ml_dtypes.bfloat16

_NC_CACHE = {}


def _icw(ic):
    return min(P, ICOL - ic * P)


def build(phases="full"):
    nc = bacc.Bacc(None, target_bir_lowering=False)
    _build(nc, phases)
    nc.finalize()
    return nc


def _build(nc, phases):
    dp = nc.declare_dram_parameter
    orig_s = dp("orig_s", [TOKS, D], F32, isOutput=False)
    post_s = dp("post_s", [TOKS, D], F32, isOutput=False)
    prior_s = dp("prior_s", [TOKS, D], F32, isOutput=False)
    hidden = dp("hidden", [BT, D], F32, isOutput=False)
    cs_cat = dp("cs_cat", [BT, 2 * HD], BF16, isOutput=False)  # [cos | sinm]
    qw_s = dp("qw_s", [D, HPC * HD], BF16, isOutput=False)
    kw_s = dp("kw_s", [D, HPC * HD], BF16, isOutput=False)
    vw_s = dp("vw_s", [D, HPC * HD], BF16, isOutput=False)
    ow_s = dp("ow_s", [HPC * HD, D], BF16, isOutput=False)
    ln1w = dp("ln1w", [D, 1], F32, isOutput=False)
    ln2w = dp("ln2w", [D, 1], F32, isOutput=False)
    gatew_s = dp("gatew_s", [D, ICOL], BF16, isOutput=False)
    upw_s = dp("upw_s", [D, ICOL], BF16, isOutput=False)
    downw_s = dp("downw_s", [ICOL, D], BF16, isOutput=False)
    # cconst: [beta_cu, beta_ce, beta_ce*ce_off, i0(=c*SB), pi0(=(c%4)*SB), 0,
    #          i0row(=(c%4)*TOKS), b(=c//4)]
    cconst = dp("cconst", [1, 8], F32, isOutput=False)

    upd_out = dp("upd_out", [SB, D], F32, isOutput=True)
    x2_out = dp("x2_out", [SB, D], F32, isOutput=True)
    selidx_out = dp("selidx_out", [SB, 1], I32, isOutput=True)
    dbg = dp("dbg", [P, 16], F32, isOutput=True)

    RG = [list(range(NC))]

    with tile.TileContext(nc) as tc, ExitStack() as es:
        # -------- DRAM internals --------
        dr = es.enter_context(tc.tile_pool(name="dram", bufs=1, space="DRAM"))

        def dtile(name, shape, dtype=F32, shared=False):
            return dr.tile(shape, dtype, name=name,
                           addr_space="Shared" if shared else "Local")

        warm_in = dtile("warm_in", [1, 8])
        warm_all = dtile("warm_all", [NC, 8], shared=True)
        sc_in = dtile("sc_in", [TOKS, 1])
        sc_all = dtile("sc_all", [BT, 1], shared=True)
        mk_in = dtile("mk_in", [TOKS, 1])
        mk_all = dtile("mk_all", [BT, 1], shared=True)
        ps_d = dtile("ps_d", [BT, 1])
        cs_own = dtile("cs_own", [SB, 2 * HD], BF16)
        cs_all = dtile("cs_all", [S, 2 * HD], BF16, shared=True)
        # token-major normed activations: [own 256 slots, feature-half]
        h1t_inA = dtile("h1t_inA", [SB, HB], BF16)
        h1t_inB = dtile("h1t_inB", [SB, HB], BF16)
        h1t_allA = dtile("h1t_allA", [S, HB], BF16, shared=True)
        h1t_allB = dtile("h1t_allB", [S, HB], BF16, shared=True)
        opartA = dtile("opartA", [S, HB], BF16)
        opartB = dtile("opartB", [S, HB], BF16)
        o_rsA = dtile("o_rsA", [SB, HB], BF16)
        o_rsB = dtile("o_rsB", [SB, HB], BF16)
        h2t_inA = dtile("h2t_inA", [SB, HB], BF16)
        h2t_inB = dtile("h2t_inB", [SB, HB], BF16)
        h2t_allA = dtile("h2t_allA", [S, HB], BF16, shared=True)
        h2t_allB = dtile("h2t_allB", [S, HB], BF16, shared=True)
        mlpA = dtile("mlpA", [S, HB], BF16)
        mlpB = dtile("mlpB", [S, HB], BF16)
        mlp_rsA = dtile("mlp_rsA", [SB, HB], BF16)
        mlp_rsB = dtile("mlp_rsB", [SB, HB], BF16)

        # -------- persistent SBUF --------
        pers = es.enter_context(tc.tile_pool(name="pers", bufs=1))
        ident_bf = pers.tile([P, P], BF16)
        make_identity(nc, ident_bf[:])
        cc_sb = pers.tile([1, 8], F32)
        nc.sync.dma_start(out=cc_sb[:], in_=cconst[:])
        ccb = pers.tile([P, 8], F32)
        nc.gpsimd.partition_broadcast(ccb[:], cc_sb[:])
        col_bcu = ccb[:, 0:1]
        col_bce = ccb[:, 1:2]
        col_ceo = ccb[:, 2:3]
        col_pi0 = ccb[:, 4:5]
        col_i0row = ccb[:, 6:7]
        col_b = ccb[:, 7:8]
        ones_bf = pers.tile([P, 1], BF16)
        nc.vector.memset(ones_bf[:], 1.0)
        epst = pers.tile([P, 1], F32)
        nc.vector.memset(epst[:], EPS)
        lnw_cols = pers.tile([P, 2 * DC], F32)  # [:, 0:16]=ln1, [:,16:32]=ln2
        nc.sync.dma_start(out=lnw_cols[:, 0:DC],
                          in_=ln1w.rearrange("(d p) one -> p d one", p=P))
        nc.sync.dma_start(out=lnw_cols[:, DC:2 * DC],
                          in_=ln2w.rearrange("(d p) one -> p d one", p=P))
        # causal masks for diagonal 128x512 chunks: keep when
        # (f - p - off) >= 0, off = (jc - 4*qb)*128
        att_mask = pers.tile([P, 4, QW], BF16)
        for r in range(4):
            nc.gpsimd.memset(att_mask[:, r, :], 1.0)
            nc.gpsimd.affine_select(
                out=att_mask[:, r, :], in_=att_mask[:, r, :],
                compare_op=OP.is_ge, fill=0.0, base=-r * P,
                pattern=[[1, QW]], channel_multiplier=-1)
        # strict-upper 32x32 (k<c) with cross-batch-row block zeroed
        tri32 = pers.tile([32, 32], F32)
        nc.gpsimd.memset(tri32[:], 1.0)
        nc.gpsimd.affine_select(out=tri32[:], in_=tri32[:],
                                compare_op=OP.is_gt, fill=0.0, base=0,
                                pattern=[[1, 32]], channel_multiplier=-1)
        nc.vector.memset(tri32[0:16, 16:32], 0.0)
        dbg_t = pers.tile([P, 16], F32)
        nc.vector.memset(dbg_t[:], 0.0)

        s_cols = [pers.tile([P, 1], F32, name=f"s_col{t}") for t in range(4)]
        m_cols = [pers.tile([P, 1], F32, name=f"m_col{t}") for t in range(4)]

        # warm up the collective rings with a tiny AllGather ASAP
        wt = pers.tile([1, 8], F32)
        nc.vector.memset(wt[:], 1.0)
        nc.sync.dma_start(out=warm_in[:], in_=wt[:])
        nc.gpsimd.collective_compute("AllGather", OP.bypass, replica_groups=RG,
                                     ins=[warm_in[:]], outs=[warm_all[:]])

        # precompute rank-phase jlt tiles (independent of scores)
        esR = ExitStack()
        rpre = esR.enter_context(tc.tile_pool(name="rankpre", bufs=1))
        iota_jmp = rpre.tile([P, T], F32, name="iota_jmp")  # value = j - p
        _it2 = rpre.tile([P, T], I32, name="iota_jmp_i")
        nc.gpsimd.iota(_it2[:], pattern=[[1, T]], base=0,
                       channel_multiplier=-1)
        nc.vector.tensor_copy(iota_jmp[:], _it2[:])
        jlt4 = rpre.tile([P, 4, T], F32, name="jlt4")
        for t in range(4):
            rhs = rpre.tile([P, 1], F32, name="jl_rhs")
            nc.vector.tensor_scalar(rhs[:], col_i0row, float(t * P - 1),
                                    None, op0=OP.add)
            nc.vector.tensor_scalar(jlt4[:, t, :], iota_jmp[:], rhs[:, :1],
                                    None, op0=OP.is_le)

        # ============ Phase R1: scores for own 512 tokens ============
        with tc.tile_pool(name="router", bufs=2) as rp:
            for t in range(4):
                cu = rp.tile([P, 1], F32, name="cu")
                ce = rp.tile([P, 1], F32, name="ce")
                for (a_ap, b_ap, dst) in ((orig_s, post_s, cu),
                                          (post_s, prior_s, ce)):
                    at = rp.tile([P, D], F32, name="r_at")
                    bt = rp.tile([P, D], F32, name="r_bt")
                    nc.sync.dma_start(out=at[:], in_=a_ap[t * P:(t + 1) * P, :])
                    nc.sync.dma_start(out=bt[:], in_=b_ap[t * P:(t + 1) * P, :])
                    df = rp.tile([P, D], F32, name="r_df")
                    nc.vector.tensor_sub(df[:], at[:], bt[:])
                    sq = rp.tile([P, D], F32, name="r_sq")
                    ssq = rp.tile([P, 1], F32, name="r_ssq")
                    nc.scalar.activation(sq[:], df[:], AF.Square,
                                         accum_out=ssq[:])
                    nc.scalar.activation(dst[:], ssq[:], AF.Sqrt)
                t1 = rp.tile([P, 1], F32, name="r_t1")
                nc.vector.tensor_scalar(t1[:], cu[:], col_bcu, None,
                                        op0=OP.mult)
                nc.vector.scalar_tensor_tensor(
                    s_cols[t][:], in0=ce[:], scalar=col_bce, in1=t1[:],
                    op0=OP.mult, op1=OP.add)
                nc.vector.tensor_scalar(s_cols[t][:], s_cols[t][:], col_ceo,
                                        None, op0=OP.add)
            sc_flat = rp.tile([P, 4], F32, name="scflat")
            for t in range(4):
                nc.vector.tensor_copy(sc_flat[:, t:t + 1], s_cols[t][:])
            nc.sync.dma_start(
                out=sc_in.rearrange("(t p) one -> p t one", p=P),
                in_=sc_flat[:])
        nc.gpsimd.collective_compute("AllGather", OP.bypass, replica_groups=RG,
                                     ins=[sc_in[:]], outs=[sc_all[:]])

        # ============ Phase R2: rank -> mask for own tokens ============
        # rank_i = #{j: s_j>s_i} + #{j<i: s_j==s_i}; mask = rank <= K-1
        # <=> acc = sum(le) - sum(eq*jlt) >= T-K+1
        with tc.tile_pool(name="rank1", bufs=1) as rp1, \
             tc.tile_pool(name="rank", bufs=2) as rp:
            sbr = rp1.tile([P, T], F32, name="sbr")
            _row_select_bcast(nc, rp1, sc_all, col_b, sbr)
            for t in range(4):
                jlt = jlt4[:, t, :]
                le = rp.tile([P, T], F32, name="k_le")
                nc.vector.tensor_scalar(le[:], sbr[:], s_cols[t][:, :1], None,
                                        op0=OP.is_le)
                eq = rp.tile([P, T], F32, name="k_eq")
                nc.vector.tensor_scalar(eq[:], sbr[:], s_cols[t][:, :1], None,
                                        op0=OP.is_equal)
                eqlt = rp.tile([P, T], F32, name="k_eqlt")
                nc.vector.tensor_mul(eqlt[:], eq[:], jlt)
                dif = rp.tile([P, T], F32, name="k_dif")
                nc.vector.tensor_sub(dif[:], le[:], eqlt[:])
                acc = rp.tile([P, 1], F32, name="k_acc")
                nc.vector.tensor_reduce(acc[:], dif[:],
                                        axis=mybir.AxisListType.X, op=OP.add)
                nacc = rp.tile([P, 1], F32, name="k_nacc")
                nc.vector.tensor_scalar_mul(nacc[:], acc[:], -1.0)
                nc.vector.tensor_scalar(m_cols[t][:], nacc[:],
                                        float(-(T - K + 1)), None,
                                        op0=OP.is_le)
            mflat = rp.tile([P, 4], F32, name="mflat")
            for t in range(4):
                nc.vector.tensor_copy(mflat[:, t:t + 1], m_cols[t][:])
            nc.sync.dma_start(
                out=mk_in.rearrange("(t p) one -> p t one", p=P), in_=mflat[:])
        nc.gpsimd.collective_compute("AllGather", OP.bypass, replica_groups=RG,
                                     ins=[mk_in[:]], outs=[mk_all[:]])
        esR.close()

        # ============ Phase R3: positions for ALL tokens (local) ============
        # layout [32 chunks (partition), 128 tokens (free)]; exclusive prefix
        # within chunk by shift+doubling; chunk offsets via tri32 matmul.
        with tc.tile_pool(name="pos", bufs=1) as pp, \
             tc.tile_pool(name="posp", bufs=1, space="PSUM") as ppp:
            mk_c = pp.tile([32, P], F32, name="mk_c")
            nc.sync.dma_start(out=mk_c[:],
                              in_=mk_all.rearrange("(c q) one -> c (q one)",
                                                   c=32))
            exA = pp.tile([32, P], F32, name="exA")
            exB = pp.tile([32, P], F32, name="exB")
            nc.vector.memset(exA[:, 0:1], 0.0)
            nc.vector.tensor_copy(exA[:, 1:P], mk_c[:, 0:P - 1])
            cur, nxt = exA, exB
            k = 1
            while k < P:
                nc.vector.tensor_copy(nxt[:, 0:k], cur[:, 0:k])
                nc.vector.tensor_add(nxt[:, k:P], cur[:, k:P], cur[:, 0:P - k])
                cur, nxt = nxt, cur
                k *= 2
            tot_col = pp.tile([32, 1], F32, name="tot_col")
            nc.vector.tensor_add(tot_col[:], cur[:, P - 1:P],
                                 mk_c[:, P - 1:P])
            ps_off = ppp.tile([32, 1], F32, space="PSUM", name="ps_off")
            nc.tensor.matmul(ps_off[:], tri32[:], tot_col[:], start=True,
                             stop=True)
            off_sb = pp.tile([32, 1], F32, name="off_sb")
            nc.vector.tensor_copy(off_sb[:], ps_off[:])
            pos_c = pp.tile([32, P], F32, name="pos_c")
            nc.vector.tensor_scalar(pos_c[:], cur[:], off_sb[:, :1], None,
                                    op0=OP.add)
            nc.sync.dma_start(
                out=ps_d.rearrange("(c q) one -> c (q one)", c=32),
                in_=pos_c[:])

        # ============ Phase OWN: own packed rows by position matching =======
        # own slot p (global c*SB+half*128+p) in batch row b has target
        # position pi = pi0 + half*128 + p within the row; its source token j
        # is the unique j with mask[j]=1 and pos[j]=pi. row = sum_j j*eq*mask.
        gpL = es.enter_context(tc.tile_pool(name="gpL", bufs=1))   # long-lived
        own_rows = []
        selh = []
        gate_g = []
        with tc.tile_pool(name="own", bufs=1) as wp:
            mbr = wp.tile([P, T], F32, name="w_mbr")
            _row_select_bcast(nc, wp, mk_all, col_b, mbr)
            pbr = wp.tile([P, T], F32, name="w_pbr")
            _row_select_bcast(nc, wp, ps_d, col_b, pbr)
            iota_j = wp.tile([P, T], F32, name="w_iotaj")
            _ij = wp.tile([P, T], I32, name="w_iotaji")
            nc.gpsimd.iota(_ij[:], pattern=[[1, T]], base=0,
                           channel_multiplier=0)
            nc.vector.tensor_copy(iota_j[:], _ij[:])
            iota_p = wp.tile([P, 1], I32, name="w_iotap")
            nc.gpsimd.iota(iota_p[:], pattern=[[0, 1]], base=0,
                           channel_multiplier=1)
            pif = wp.tile([P, 1], F32, name="w_pif")
            nc.vector.tensor_copy(pif[:], iota_p[:])
            nc.vector.tensor_scalar(pif[:], pif[:], col_pi0, None, op0=OP.add)
            for half in range(2):
                pih = wp.tile([P, 1], F32, name="w_pih")
                nc.vector.tensor_scalar(pih[:], pif[:], float(half * P), None,
                                        op0=OP.add)
                eq = wp.tile([P, T], F32, name="w_eq")
                nc.vector.tensor_scalar(eq[:], pbr[:], pih[:, :1], None,
                                        op0=OP.is_equal)
                nc.vector.tensor_mul(eq[:], eq[:], mbr[:])
                nc.vector.tensor_mul(eq[:], eq[:], iota_j[:])
                rowf = wp.tile([P, 1], F32, name="w_rowf")
                nc.vector.tensor_reduce(rowf[:], eq[:],
                                        axis=mybir.AxisListType.X, op=OP.add)
                # flat row = b*T + j
                nc.vector.tensor_scalar(rowf[:], col_b, float(T), rowf[:, :1],
                                        op0=OP.mult, op1=OP.add)
                orow = gpL.tile([P, 1], I32, name=f"orow{half}")
                nc.vector.tensor_copy(orow[:], rowf[:])
                own_rows.append(orow)

        # ============ Phase G: gathers ============
        for half in range(2):
            sh = gpL.tile([P, D], F32, name=f"selh{half}")
            nc.gpsimd.indirect_dma_start(
                out=sh[:], out_offset=None, in_=hidden[:],
                in_offset=bass.IndirectOffsetOnAxis(
                    ap=own_rows[half][:, :1], axis=0),
                bounds_check=BT - 1, oob_is_err=False)
            selh.append(sh)
        # gather own slots' cos|sinm rows; AllGather to all cores
        for half in range(2):
            csh = gpL.tile([P, 2 * HD], BF16, name=f"csh{half}")
            nc.gpsimd.indirect_dma_start(
                out=csh[:], out_offset=None, in_=cs_cat[:],
                in_offset=bass.IndirectOffsetOnAxis(
                    ap=own_rows[half][:, :1], axis=0))
            nc.sync.dma_start(out=cs_own[half * P:(half + 1) * P, :],
                              in_=csh[:])
        nc.gpsimd.collective_compute("AllGather", OP.bypass, replica_groups=RG,
                                     ins=[cs_own[:]], outs=[cs_all[:]])
        for half in range(2):
            ssc = gpL.tile([P, 1], F32, name=f"ssc{half}")
            nc.gpsimd.indirect_dma_start(
                out=ssc[:], out_offset=None, in_=sc_all[:],
                in_offset=bass.IndirectOffsetOnAxis(
                    ap=own_rows[half][:, :1], axis=0))
            gg = gpL.tile([P, 1], F32, name=f"gate{half}")
            nc.scalar.activation(gg[:], ssc[:], AF.Sigmoid)
            gate_g.append(gg)
        x1 = [gpL.tile([P, D], F32, name=f"x1_{i}") for i in range(2)]

        if phases != "full":
            with tc.tile_pool(name="rfin", bufs=2) as fp:
                for half in range(2):
                    nc.sync.dma_start(
                        out=upd_out[half * P:(half + 1) * P, :],
                        in_=selh[half][:])
                    nc.sync.dma_start(
                        out=x2_out[half * P:(half + 1) * P, :],
                        in_=selh[half][:])
                    nc.sync.dma_start(out=selidx_out[half * P:(half + 1) * P, :],
                                      in_=own_rows[half][:])
                nc.vector.tensor_copy(dbg_t[:, 5:6], gate_g[0][:])
                nc.sync.dma_start(out=dbg[:], in_=dbg_t[:])
            return

        # ============ Phase N1: h1 = rmsnorm(selh); token-major AG ==========
        # (ln1 weight is folded in on the consumer side, after transpose)
        with tc.tile_pool(name="n1", bufs=2) as np_:
            sq_scr = np_.tile([P, D], F32, name="sq_scr1")
            for half in range(2):
                h1b = np_.tile([P, D], BF16, name="h1b")
                _rmsnorm_bf(nc, np_, selh[half], h1b, sq_scr, epst)
                sl = slice(half * P, (half + 1) * P)
                nc.sync.dma_start(out=h1t_inA[sl, :], in_=h1b[:, 0:HB])
                nc.sync.dma_start(out=h1t_inB[sl, :], in_=h1b[:, HB:D])
        nc.gpsimd.collective_compute("AllGather", OP.bypass, replica_groups=RG,
                                     ins=[h1t_inA[:]], outs=[h1t_allA[:]])
        nc.gpsimd.collective_compute("AllGather", OP.bypass, replica_groups=RG,
                                     ins=[h1t_inB[:]], outs=[h1t_allB[:]])

        # attention-lived pool (qh/kh/vtok/o_fm survive into OPROJ)
        esA = ExitStack()
        gpA = esA.enter_context(tc.tile_pool(name="gpA", bufs=1))
        qh = [gpA.tile([P, S], BF16, name=f"qh{h}") for h in range(HPC)]
        kh = [gpA.tile([P, S], BF16, name=f"kh{h}") for h in range(HPC)]
        vtok = [gpA.tile([P, S // P, HD], BF16, name=f"vtok{h}")
                for h in range(HPC)]
        o_fm = [gpA.tile([P, S], BF16, name=f"ofm{h}") for h in range(HPC)]
        ow_sb = gpA.tile([P, HPC, D], BF16, name="ow_sb")
        for h in range(HPC):
            nc.sync.dma_start(out=ow_sb[:, h, :],
                              in_=ow_s[h * P:(h + 1) * P, :])

        # ============ Phase QKV (own 2 heads, full S) ============
        esQ = ExitStack()
        gpQ = esQ.enter_context(tc.tile_pool(name="gpQ", bufs=1))
        h1T = gpQ.tile([P, DC, S], BF16, name="h1T")
        qw_sb = gpQ.tile([P, DC, HPC * HD], BF16, name="qw_sb")
        kw_sb = gpQ.tile([P, DC, HPC * HD], BF16, name="kw_sb")
        vw_sb = gpQ.tile([P, DC, HPC * HD], BF16, name="vw_sb")
        for (wsb, wsrc) in ((qw_sb, qw_s), (kw_sb, kw_s), (vw_sb, vw_s)):
            for d in range(DC):
                nc.scalar.dma_start(out=wsb[:, d, :],
                                    in_=wsrc[d * P:(d + 1) * P, :])
        # cos|sinm for all S slots (from AG), transposed to feature-major
        cosT = gpQ.tile([P, S], BF16, name="cosT")
        sinmT = gpQ.tile([P, S], BF16, name="sinmT")
        with tc.tile_pool(name="csg", bufs=3) as cp, \
             tc.tile_pool(name="csgp", bufs=4, space="PSUM") as cpp:
            for sc_ in range(S // P):
                csg = cp.tile([P, 2 * HD], BF16, name="csg")
                nc.sync.dma_start(out=csg[:],
                                  in_=cs_all[sc_ * P:(sc_ + 1) * P, :])
                for (lo, dstT) in ((0, cosT), (HD, sinmT)):
                    pt = cpp.tile([P, P], BF16, space="PSUM", name="cs_p")
                    nc.tensor.transpose(pt[:], csg[:, lo:lo + HD], ident_bf[:])
                    nc.vector.tensor_copy(dstT[:, sc_ * P:(sc_ + 1) * P],
                                          pt[:])

        with tc.tile_pool(name="qkv", bufs=3) as qp, \
             tc.tile_pool(name="qkvtk", bufs=3) as qtk, \
             tc.tile_pool(name="qkvp", bufs=2, space="PSUM") as qpp, \
             tc.tile_pool(name="qkvtp", bufs=2, space="PSUM") as qtp:
            for n in range(NQ):
                # transpose 4 token-major s-chunks into h1T (ln1 folded)
                for sc_ in range(4 * n, 4 * n + 4):
                    for (src, dlo) in ((h1t_allA, 0), (h1t_allB, 8)):
                        tok = qtk.tile([P, HB], BF16, name="tok")
                        nc.sync.dma_start(
                            out=tok[:], in_=src[sc_ * P:(sc_ + 1) * P, :])
                        for dd in range(8):
                            d = dlo + dd
                            pt = qtp.tile([P, P], BF16, space="PSUM",
                                          name="tk_p")
                            nc.tensor.transpose(
                                pt[:], tok[:, dd * P:(dd + 1) * P], ident_bf[:])
                            nc.vector.tensor_scalar(
                                h1T[:, d, sc_ * P:(sc_ + 1) * P], pt[:],
                                lnw_cols[:, d:d + 1], None, op0=OP.mult)
                for h in range(HPC):
                    for (wsb, dsth) in ((qw_sb, qh), (kw_sb, kh)):
                        pt = qpp.tile([P, QW], F32, space="PSUM", name="qk_ps")
                        for d in range(DC):
                            nc.tensor.matmul(
                                pt[:], wsb[:, d, h * HD:(h + 1) * HD],
                                h1T[:, d, n * QW:(n + 1) * QW],
                                start=(d == 0), stop=(d == DC - 1))
                        # rope: out = pt*cos + rot(pt)*sinm
                        rot = qp.tile([P, QW], F32, name="rp_rot")
                        nc.vector.tensor_copy(rot[0:64, :], pt[64:P, :])
                        nc.vector.tensor_copy(rot[64:P, :], pt[0:64, :])
                        t1 = qp.tile([P, QW], F32, name="rp_t1")
                        nc.vector.tensor_mul(
                            t1[:], pt[:], cosT[:, n * QW:(n + 1) * QW])
                        t2 = qp.tile([P, QW], F32, name="rp_t2")
                        nc.vector.tensor_mul(
                            t2[:], rot[:], sinmT[:, n * QW:(n + 1) * QW])
                        nc.vector.tensor_add(
                            dsth[h][:, n * QW:(n + 1) * QW], t1[:], t2[:])
                    pt = qpp.tile([P, QW], F32, space="PSUM", name="v_ps")
                    for d in range(DC):
                        nc.tensor.matmul(
                            pt[:], vw_sb[:, d, h * HD:(h + 1) * HD],
                            h1T[:, d, n * QW:(n + 1) * QW],
                            start=(d == 0), stop=(d == DC - 1))
                    vsb = qp.tile([P, QW], BF16, name="v_sb")
                    nc.vector.tensor_copy(vsb[:], pt[:])
                    for kk in range(4):
                        ptt = qtp.tile([P, P], BF16, space="PSUM", name="vt_ps")
                        nc.tensor.transpose(ptt[:], vsb[:, kk * P:(kk + 1) * P],
                                            ident_bf[:])
                        nc.vector.tensor_copy(vtok[h][:, n * 4 + kk, :],
                                              ptt[:])
        esQ.close()

        # ============ Phase ATT (own heads, causal, full S queries) ========
        with tc.tile_pool(name="att", bufs=4) as ap, \
             tc.tile_pool(name="attpa", bufs=3, space="PSUM") as apa, \
             tc.tile_pool(name="attpo", bufs=2, space="PSUM") as apo:
            for h in range(HPC):
                for qb in range(NQ):
                    jmax = 4 * (qb + 1)
                    po = apo.tile([P, QW], F32, space="PSUM", name="a_po")
                    psum = apo.tile([1, QW], F32, space="PSUM", name="a_ps")
                    for jc in range(jmax):
                        pa = apa.tile([P, QW], F32, space="PSUM", name="a_pa")
                        nc.tensor.matmul(pa[:], kh[h][:, jc * P:(jc + 1) * P],
                                         qh[h][:, qb * QW:(qb + 1) * QW],
                                         start=True, stop=True)
                        et = ap.tile([P, QW], BF16, name="a_et")
                        nc.scalar.activation(et[:], pa[:], AF.Exp, scale=SCALE)
                        if jc >= 4 * qb:
                            nc.vector.tensor_mul(et[:], et[:],
                                                 att_mask[:, jc - 4 * qb, :])
                        nc.tensor.matmul(psum[:], ones_bf[:], et[:],
                                         start=(jc == 0), stop=(jc == jmax - 1),
                                         skip_group_check=True)
                        nc.tensor.matmul(po[:], vtok[h][:, jc, :], et[:],
                                         start=(jc == 0), stop=(jc == jmax - 1),
                                         skip_group_check=True)
                    rec = ap.tile([1, QW], F32, name="a_rec")
                    nc.vector.reciprocal(rec[:], psum[:])
                    recb = ap.tile([P, QW], F32, name="a_recb")
                    nc.gpsimd.partition_broadcast(recb[:], rec[:])
                    nc.vector.tensor_mul(o_fm[h][:, qb * QW:(qb + 1) * QW],
                                         po[:], recb[:])

        # ============ Phase OPROJ: opart[s,d] = sum_h o_fm_h.T @ ow_h ======
        with tc.tile_pool(name="opj", bufs=3) as op_, \
             tc.tile_pool(name="opjp", bufs=3, space="PSUM") as opp:
            for nd in range(NQ):
                dstpart = opartA if nd < 2 else opartB
                dlo = (nd % 2) * QW
                for qc in range(S // P):
                    pt = opp.tile([P, QW], F32, space="PSUM", name="o_ps")
                    for h in range(HPC):
                        nc.tensor.matmul(
                            pt[:], o_fm[h][:, qc * P:(qc + 1) * P],
                            ow_sb[:, h, nd * QW:(nd + 1) * QW],
                            start=(h == 0), stop=(h == HPC - 1))
                    osb = op_.tile([P, QW], BF16, name="o_sb")
                    nc.vector.tensor_copy(osb[:], pt[:])
                    nc.sync.dma_start(
                        out=dstpart[qc * P:(qc + 1) * P, dlo:dlo + QW],
                        in_=osb[:])
                if nd == 1:
                    nc.gpsimd.collective_compute(
                        "ReduceScatter", OP.add, replica_groups=RG,
                        ins=[opartA[:]], outs=[o_rsA[:]])
            nc.gpsimd.collective_compute(
                "ReduceScatter", OP.add, replica_groups=RG,
                ins=[opartB[:]], outs=[o_rsB[:]])
        esA.close()

        # MLP weights (loads overlap RS_o / N2 / AG_h2)
        esM = ExitStack()
        gpMw = esM.enter_context(tc.tile_pool(name="gpMw", bufs=1))
        gw_sb = gpMw.tile([P, DC, ICOL], BF16, name="gw_sb")
        uw_sb = gpMw.tile([P, DC, ICOL], BF16, name="uw_sb")
        dw_sb = gpMw.tile([P, NIC, D], BF16, name="dw_sb")
        for (wsb, wsrc) in ((gw_sb, gatew_s), (uw_sb, upw_s)):
            for d in range(DC):
                nc.scalar.dma_start(out=wsb[:, d, :],
                                    in_=wsrc[d * P:(d + 1) * P, :])
        for ic in range(NIC):
            icw = _icw(ic)
            nc.scalar.dma_start(out=dw_sb[0:icw, ic, :],
                                in_=downw_s[ic * P:ic * P + icw, :])

        # ============ Phase N2 + AG (token-major) ============
        with tc.tile_pool(name="n2", bufs=2) as np2:
            sq_scr = np2.tile([P, D], F32, name="sq_scr2")
            for half in range(2):
                sl = slice(half * P, (half + 1) * P)
                orsa = np2.tile([P, HB], BF16, name="orsa")
                orsb = np2.tile([P, HB], BF16, name="orsb")
                nc.sync.dma_start(out=orsa[:], in_=o_rsA[sl, :])
                nc.sync.dma_start(out=orsb[:], in_=o_rsB[sl, :])
                nc.vector.tensor_add(x1[half][:, 0:HB], selh[half][:, 0:HB],
                                     orsa[:])
                nc.vector.tensor_add(x1[half][:, HB:D],
                                     selh[half][:, HB:D], orsb[:])
                h2b = np2.tile([P, D], BF16, name="h2b")
                _rmsnorm_bf(nc, np2, x1[half], h2b, sq_scr, epst)
                nc.sync.dma_start(out=h2t_inA[sl, :], in_=h2b[:, 0:HB])
                nc.sync.dma_start(out=h2t_inB[sl, :], in_=h2b[:, HB:D])
        nc.gpsimd.collective_compute("AllGather", OP.bypass, replica_groups=RG,
                                     ins=[h2t_inA[:]], outs=[h2t_allA[:]])
        nc.gpsimd.collective_compute("AllGather", OP.bypass, replica_groups=RG,
                                     ins=[h2t_inB[:]], outs=[h2t_allB[:]])

        # ============ Phase MLP (TP over I) ============
        gpMa = esM.enter_context(tc.tile_pool(name="gpMa", bufs=1))
        h2T = gpMa.tile([P, DC, S], BF16, name="h2T")
        act_sb = gpMa.tile([P, NIC, S], BF16, name="act_sb")
        with tc.tile_pool(name="mlp", bufs=3) as mp, \
             tc.tile_pool(name="mlptk", bufs=3) as mtk, \
             tc.tile_pool(name="mlpp", bufs=2, space="PSUM") as mpp, \
             tc.tile_pool(name="mlptp", bufs=2, space="PSUM") as mtp:
            for n in range(NQ):
                for sc_ in range(4 * n, 4 * n + 4):
                    for (src, dlo) in ((h2t_allA, 0), (h2t_allB, 8)):
                        tok = mtk.tile([P, HB], BF16, name="tok2")
                        nc.sync.dma_start(
                            out=tok[:], in_=src[sc_ * P:(sc_ + 1) * P, :])
                        for dd in range(8):
                            d = dlo + dd
                            pt = mtp.tile([P, P], BF16, space="PSUM",
                                          name="tk2_p")
                            nc.tensor.transpose(
                                pt[:], tok[:, dd * P:(dd + 1) * P], ident_bf[:])
                            nc.vector.tensor_scalar(
                                h2T[:, d, sc_ * P:(sc_ + 1) * P], pt[:],
                                lnw_cols[:, DC + d:DC + d + 1], None,
                                op0=OP.mult)
                for ic in range(NIC):
                    icw = _icw(ic)
                    pg = mpp.tile([P, QW], F32, space="PSUM", name="m_pg")
                    pu = mpp.tile([P, QW], F32, space="PSUM", name="m_pu")
                    for d in range(DC):
                        nc.tensor.matmul(pg[0:icw, :],
                                         gw_sb[:, d, ic * P:ic * P + icw],
                                         h2T[:, d, n * QW:(n + 1) * QW],
                                         start=(d == 0), stop=(d == DC - 1))
                    for d in range(DC):
                        nc.tensor.matmul(pu[0:icw, :],
                                         uw_sb[:, d, ic * P:ic * P + icw],
                                         h2T[:, d, n * QW:(n + 1) * QW],
                                         start=(d == 0), stop=(d == DC - 1))
                    sg = mp.tile([P, QW], BF16, name="m_sg")
                    nc.scalar.activation(sg[0:icw, :], pg[0:icw, :], AF.Silu)
                    nc.vector.tensor_mul(
                        act_sb[0:icw, ic, n * QW:(n + 1) * QW],
                        sg[0:icw, :], pu[0:icw, :])
            # down proj: nd-outer; ReduceScatter of first half overlaps rest
            for nd in range(NQ):
                dstpart = mlpA if nd < 2 else mlpB
                dlo = (nd % 2) * QW
                for sc_ in range(S // P):
                    pt = mpp.tile([P, QW], F32, space="PSUM", name="m_pd")
                    for ic in range(NIC):
                        icw = _icw(ic)
                        nc.tensor.matmul(
                            pt[:], act_sb[0:icw, ic, sc_ * P:(sc_ + 1) * P],
                            dw_sb[0:icw, ic, nd * QW:(nd + 1) * QW],
                            start=(ic == 0), stop=(ic == NIC - 1))
                    msb = mp.tile([P, QW], BF16, name="m_sb")
                    nc.vector.tensor_copy(msb[:], pt[:])
                    nc.sync.dma_start(
                        out=dstpart[sc_ * P:(sc_ + 1) * P, dlo:dlo + QW],
                        in_=msb[:])
                if nd == 1:
                    nc.gpsimd.collective_compute(
                        "ReduceScatter", OP.add, replica_groups=RG,
                        ins=[mlpA[:]], outs=[mlp_rsA[:]])
            nc.gpsimd.collective_compute(
                "ReduceScatter", OP.add, replica_groups=RG,
                ins=[mlpB[:]], outs=[mlp_rsB[:]])
        esM.close()

        # ============ Final ============
        with tc.tile_pool(name="fin", bufs=2) as fp:
            for half in range(2):
                sl = slice(half * P, (half + 1) * P)
                mta = fp.tile([P, HB], BF16, name="f_mta")
                mtb = fp.tile([P, HB], BF16, name="f_mtb")
                nc.sync.dma_start(out=mta[:], in_=mlp_rsA[sl, :])
                nc.sync.dma_start(out=mtb[:], in_=mlp_rsB[sl, :])
                x2 = fp.tile([P, D], F32, name="f_x2")
                nc.vector.tensor_add(x2[:, 0:HB], x1[half][:, 0:HB], mta[:])
                nc.vector.tensor_add(x2[:, HB:D], x1[half][:, HB:D], mtb[:])
                nc.sync.dma_start(out=x2_out[sl, :], in_=x2[:])
                dlt = fp.tile([P, D], F32, name="f_dlt")
                nc.vector.tensor_sub(dlt[:], x2[:], selh[half][:])
                upd = fp.tile([P, D], F32, name="f_upd")
                nc.vector.scalar_tensor_tensor(
                    upd[:], in0=dlt[:], scalar=gate_g[half][:, :1],
                    in1=selh[half][:], op0=OP.mult, op1=OP.add)
                nc.sync.dma_start(out=upd_out[sl, :], in_=upd[:])
                nc.sync.dma_start(out=selidx_out[sl, :],
                                  in_=own_rows[half][:])
            nc.vector.tensor_copy(dbg_t[:, 8:9], gate_g[0][:])
            nc.sync.dma_start(out=dbg[:], in_=dbg_t[:])


def _rmsnorm_bf(nc, pool, x, out_bf, sq_scr, epst):
    """out_bf = bf16(x * rsqrt(mean(x^2)+eps)), x f32 [128, D]."""
    ssq = pool.tile([P, 1], F32, name="rn_ssq")
    nc.scalar.activation(sq_scr[:], x[:], AF.Square, accum_out=ssq[:])
    rt = pool.tile([P, 1], F32, name="rn_rt")
    nc.scalar.activation(rt[:], ssq[:], AF.Sqrt, scale=1.0 / D,
                         bias=epst[:, :1])
    rec = pool.tile([P, 1], F32, name="rn_rec")
    nc.vector.reciprocal(rec[:], rt[:])
    nc.scalar.activation(out_bf[:], x[:], AF.Copy, scale=rec[:, :1])


def _row_select_bcast(nc, pool, src_all, col_b, out_bcast):
    """out = broadcast(src_all row-block b), b in {0,1} from col_b."""
    r0 = pool.tile([1, T], F32, name="rs_r0")
    r1 = pool.tile([1, T], F32, name="rs_r1")
    v = src_all.rearrange("(a t) one -> a (t one)", a=2)
    nc.sync.dma_start(out=r0[:], in_=v[0:1, :])
    nc.sync.dma_start(out=r1[:], in_=v[1:2, :])
    b0 = pool.tile([P, T], F32, name="rs_b0")
    b1 = pool.tile([P, T], F32, name="rs_b1")
    nc.gpsimd.partition_broadcast(b0[:], r0[:])
    nc.gpsimd.partition_broadcast(b1[:], r1[:])
    df = pool.tile([P, T], F32, name="rs_df")
    nc.vector.tensor_sub(df[:], b1[:], b0[:])
    nc.vector.scalar_tensor_tensor(out_bcast[:], in0=df[:], scalar=col_b,
                                   in1=b0[:], op0=OP.mult, op1=OP.add)


# =====================================================================
# Host side
# =====================================================================
def kernel(**inputs):
    hs = np.asarray(inputs["hidden_states"], np.float32)
    qw = np.asarray(inputs["q_w"], np.float32)
    kw = np.asarray(inputs["k_w"], np.float32)
    vw = np.asarray(inputs["v_w"], np.float32)
    ow = np.asarray(inputs["o_w"], np.float32)
    bcu = float(np.asarray(inputs["beta_cu"]))
    bce = float(np.asarray(inputs["beta_ce"]))
    ceo = float(np.asarray(inputs["ce_off"]))

    hs_f = np.ascontiguousarray(hs.reshape(BT, D))
    orig_f = np.asarray(inputs["original"], np.float32).reshape(BT, D)
    post_f = np.asarray(inputs["posterior"], np.float32).reshape(BT, D)
    prior_f = np.asarray(inputs["prior"], np.float32).reshape(BT, D)
    cos_f = np.asarray(inputs["cos"], np.float32).reshape(BT, HD)
    sin_f = np.asarray(inputs["sin"], np.float32).reshape(BT, HD)
    sinm = sin_f.copy()
    sinm[:, : HD // 2] = -sinm[:, : HD // 2]
    cs_cat = np.ascontiguousarray(
        np.concatenate([cos_f, sinm], axis=1)).astype(BF16_NP)

    gw = np.asarray(inputs["gate_w"], np.float32)
    uw = np.asarray(inputs["up_w"], np.float32)
    dw = np.asarray(inputs["down_w"], np.float32)

    in_maps = []
    for c in range(NC):
        sl = slice(c * TOKS, (c + 1) * TOKS)
        hd_sl = slice(c * HPC * HD, (c + 1) * HPC * HD)
        ic_sl = slice(c * ICOL, (c + 1) * ICOL)
        b = c // 4
        cconst = np.array([[bcu, bce, bce * ceo, c * SB, (c % 4) * SB,
                            0.0, (c % 4) * TOKS, b]], np.float32)
        in_maps.append({
            "orig_s": np.ascontiguousarray(orig_f[sl]),
            "post_s": np.ascontiguousarray(post_f[sl]),
            "prior_s": np.ascontiguousarray(prior_f[sl]),
            "hidden": hs_f,
            "cs_cat": cs_cat,
            "qw_s": np.ascontiguousarray(qw[:, hd_sl]).astype(BF16_NP),
            "kw_s": np.ascontiguousarray(kw[:, hd_sl]).astype(BF16_NP),
            "vw_s": np.ascontiguousarray(vw[:, hd_sl]).astype(BF16_NP),
            "ow_s": np.ascontiguousarray(ow[hd_sl, :]).astype(BF16_NP),
            "ln1w": np.asarray(inputs["ln1_w"], np.float32).reshape(-1, 1),
            "ln2w": np.asarray(inputs["ln2_w"], np.float32).reshape(-1, 1),
            "gatew_s": np.ascontiguousarray(gw[:, ic_sl]).astype(BF16_NP),
            "upw_s": np.ascontiguousarray(uw[:, ic_sl]).astype(BF16_NP),
            "downw_s": np.ascontiguousarray(dw[ic_sl, :]).astype(BF16_NP),
            "cconst": cconst,
        })

    global _last_in_maps
    _last_in_maps = in_maps
    import os
    ph = os.environ.get("KPHASES", "full")
    if ph not in _NC_CACHE:
        _NC_CACHE[ph] = build(phases=ph)
    nc = _NC_CACHE[ph]
    res = run_bass_kernel_spmd(nc, in_maps, core_ids=list(range(NC)))

    global _last_results
    _last_results = [res.results[c] for c in range(NC)]
    out = hs_f.copy()
    for c in range(NC):
        idx = res.results[c]["selidx_out"][:, 0]
        out[idx] = res.results[c]["upd_out"]
    return out.reshape(B, T, D)


if __name__ == "__main__":
    import reference
    inp = {k: np.asarray(v) for k, v in reference.setup_inputs().items()}
    got = kernel(**inp)
    want = np.asarray(reference.reference(**reference.setup_inputs()))
    err = np.abs(got - want).max() / np.abs(want).max()
    print("rel err:", err)
